# revision 1
# baseline (speedup 1.0000x reference)
"""Trainium2 Bass kernel for 16-head causal multi-head attention.

Problem: B=2, S=2048, D=1024, H=16 (head dim 64), causal mask.
    out = softmax((XqWq+bq)(XkWk+bk)^T / 8, causal) (XvWv+bv) Wo + bo

Sharding: tensor-parallel over heads. Each of the 8 cores owns 2 heads:
Wq/Wk/Wv column-sliced (128 cols), Wo row-sliced (128 rows). Each core
computes its heads end-to-end and produces a partial output (ctx_c @ Wo_c);
the host sums the 8 partials and adds (bv @ Wo + bo).

Device-side layout choices (per core):
  - Host passes X^T (features-major) in fp16, so projection matmuls get the
    contraction dim (features) on partitions without any device transposes.
  - Q^T, K^T are produced as [128 = 2 heads x 64 dk, 4096 tokens] fp16.
  - V is produced token-major [tok, dk] (needed as PV's stationary operand),
    augmented with a ones column so each PV matmul also yields the softmax
    denominator for free.
  - Scores are computed transposed, S^T[k, q] = K @ Q^T, so that softmax'd
    probabilities P^T already have the PV contraction dim (k) on partitions.
  - exp has no max subtraction (scores/8 are ~N(0,1); exp never overflows
    fp32), so no flash-style rescaling is needed and the softmax denominator
    can come out of the PV matmul's ones column.
  - Causal structure is exploited: fully-masked k-tiles are skipped, the
    partly-masked diagonal 128x128 sub-blocks are masked by multiplying with
    a host-provided triangular 0/1 tile.
"""

import math

import numpy as np

# Full-problem constants
B, S, D, H = 2, 2048, 1024, 16
DK = D // H  # 64
NCORES = 8
HPC = H // NCORES  # heads per core
P = 128
QC = 512  # tokens per attention q-chunk / projection chunk

_PROGRAM_CACHE = {}
TRACE = False  # set True (e.g. from test.py) to capture an NTFF profile
LAST = {}      # holds the most recent BassKernelResults


# ---------------------------------------------------------------------------
# Device program
# ---------------------------------------------------------------------------

def _mha_body(ctx, tc, io, s, d, b):
    import concourse.bass as bass
    from concourse import mybir

    F16 = mybir.dt.float16
    F32 = mybir.dt.float32
    Exp = mybir.ActivationFunctionType.Exp
    Identity = mybir.ActivationFunctionType.Identity

    nc = tc.nc
    nch = s // QC       # q chunks per sequence
    kpc = QC // P       # k tiles per chunk (4)
    nf = d // P         # feature tiles

    xq, xk, xv = io["xq_t"], io["xk_t"], io["xv_t"]
    wq, wk, wv, wo = io["wq"], io["wk"], io["wv"], io["wo"]
    bq, bk = io["bq"], io["bk"]
    tri = io["tri"]
    out_t = io["out_t"]

    consts = ctx.enter_context(tc.tile_pool(name="consts", bufs=1))
    persist = ctx.enter_context(tc.tile_pool(name="persist", bufs=1))
    xs = ctx.enter_context(tc.tile_pool(name="xs", bufs=1))
    pts = ctx.enter_context(tc.tile_pool(name="pts", bufs=3))
    ptd = ctx.enter_context(tc.tile_pool(name="ptd", bufs=1))
    rcs = ctx.enter_context(tc.tile_pool(name="rcs", bufs=2))
    wout = ctx.enter_context(tc.tile_pool(name="wout", bufs=4))
    pspool = ctx.enter_context(tc.tile_pool(name="ps", bufs=1, space="PSUM"))

    # PSUM bank map (8 banks):
    #   bk0+bk1 / bk2+bk3: double-buffered 2-bank "wide" score tiles
    #     [128, 1024] = both heads' S^T for one k-tile side by side
    #   bk4 / bk5: PV accumulators (ctx + softmax sums) per head
    #   bk6: normalize broadcast
    #   bk7: projections + output projection
    def ps_tile(tag, width=QC):
        return pspool.tile([P, width], F32, tag=tag, name=tag)

    # ---- constants -------------------------------------------------------
    wq_sb = consts.tile([P, nf, P], F16, tag="wq")
    nc.sync.dma_start(wq_sb[:], wq.rearrange("p (o m) -> p o m", m=P))
    wk_sb = consts.tile([P, nf, P], F16, tag="wk")
    nc.sync.dma_start(wk_sb[:], wk.rearrange("p (o m) -> p o m", m=P))
    wv_sb = consts.tile([P, nf, P], F16, tag="wv")
    nc.sync.dma_start(wv_sb[:], wv.rearrange("p (o m) -> p o m", m=P))
    wo_sb = consts.tile([P, d], F16, tag="wo")
    nc.sync.dma_start(wo_sb[:], wo[:, :])
    tri_sb = consts.tile([P, P], F16, tag="tri")
    nc.sync.dma_start(tri_sb[:], tri[:, :])
    bq_sb = consts.tile([P, 1], F32, tag="bq")
    nc.sync.dma_start(bq_sb[:], bq[:, :])
    bk_sb = consts.tile([P, 1], F32, tag="bk")
    nc.sync.dma_start(bk_sb[:], bk[:, :])
    ones_sb = consts.tile([P, 64], F16, tag="ones")
    nc.vector.memset(ones_sb[:], 1.0)

    qt_tiles = {}
    kt_tiles = {}
    v_tiles = {}
    diag_zeroed = set()
    pending_norm_wo = None

    for bb in range(b):
        # One big DMA per (input, feature-tile): [128, s] fp16 covering the
        # whole batch sequence (minimizes per-DMA fixed costs). bufs=1 tags:
        # the bb=1 loads naturally wait for (and overlap) bb=0's consumers.
        bx = {}
        bx0 = {}
        for nm, xsrc in (("q", xq), ("k", xk), ("v", xv)):
            for f in range(nf):
                x0 = xs.tile([P, QC], F16, tag=f"c0x{nm}{f}", name=f"c0x{nm}{f}")
                nc.sync.dma_start(x0[:], xsrc[f * P:(f + 1) * P,
                                              bb * s:bb * s + QC])
                bx0[(nm, f)] = x0
        for nm, xsrc in (("q", xq), ("k", xk), ("v", xv)):
            for f in range(nf):
                xt = xs.tile([P, s - QC], F16, tag=f"x{nm}{f}", name=f"x{nm}{f}")
                nc.sync.dma_start(xt[:], xsrc[f * P:(f + 1) * P,
                                              bb * s + QC:(bb + 1) * s])
                bx[(nm, f)] = xt

        def xsl(nm, f, lo, hi):
            """Slice tokens [lo:hi) of this batch from fast-path/wide tiles."""
            if hi <= QC:
                return bx0[(nm, f)][:, lo:hi]
            return bx[(nm, f)][:, lo - QC:hi - QC]

        for jj in range(nch):  # chunk-interleaved: project j, then attend j
            j = bb * nch + jj
            co = jj * QC

            # ---- projections for chunk j (single PSUM bank bk7) ----------
            for (nm, w_sb, b_sb, store) in (
                ("q", wq_sb, bq_sb, qt_tiles),
                ("k", wk_sb, bk_sb, kt_tiles),
            ):
                pp = ps_tile("bk7")
                for f in range(nf):
                    nc.tensor.matmul(pp[:], w_sb[:, f, :],
                                     xsl(nm, f, co, co + QC),
                                     start=(f == 0), stop=(f == nf - 1))
                t = persist.tile([P, QC], F16, tag=f"{nm}t{j}")
                nc.scalar.activation(t[:], pp[:], Identity, bias=b_sb[:, 0:1],
                                     scale=1.0)
                store[j] = t

            for t4 in range(kpc):  # V: [tok, dk] per 128-token tile
                pp = ps_tile("bk7")
                for f in range(nf):
                    nc.tensor.matmul(pp[:, t4 * P:(t4 + 1) * P],
                                     xsl("v", f, co + t4 * P, co + (t4 + 1) * P),
                                     wv_sb[:, f, :],
                                     start=(f == 0), stop=(f == nf - 1))
                kt = jj * kpc + t4
                for h in range(HPC):
                    vt = persist.tile([P, 65], F16, tag=f"v{h}_{bb}_{kt}",
                                      name=f"v{h}_{bb}_{kt}")
                    nc.vector.memset(vt[:, 64:65], 1.0)
                    nc.vector.tensor_copy(
                        vt[:, 0:64], pp[:, t4 * P + h * 64:t4 * P + h * 64 + 64])
                    v_tiles[(bb, kt, h)] = vt

            # norm + Wo of the PREVIOUS chunk goes here: its reciprocal/
            # broadcast chain hides behind this chunk's projection matmuls.
            if pending_norm_wo is not None:
                pending_norm_wo()
                pending_norm_wo = None

            # ---- attention for chunk (bb, jj) ----------------------------
            # Software-pipelined emission: QK/exp of k-tile kt+1 is emitted
            # BEFORE PV of k-tile kt, so the in-order PE always has matmul
            # work while the ACT engine runs exp.
            i = jj
            ctx_t = persist.tile([P, QC], F16, tag=f"ctx{j % 2}",
                                 name=f"ctx{j % 2}")
            pc = {0: ps_tile("bk4"), 1: ps_tile("bk5")}
            nkt_i = kpc * (i + 1)
            qtile = qt_tiles[j]

            def emit_qk_exp(kt, i=i, bb=bb, qtile=qtile):
                """QK matmuls + exp for k-tile kt; returns PV emit closure."""
                jk = bb * nch + kt // kpc
                ko = (kt % kpc) * P
                tdiag = kt - kpc * i
                ktile = kt_tiles[jk]
                sw = ps_tile("swA" if kt % 2 == 0 else "swB", width=2 * QC)
                if tdiag < 0:
                    for h in range(HPC):
                        nc.tensor.matmul(sw[:, h * QC:(h + 1) * QC],
                                         ktile[h * 64:h * 64 + 64, ko:ko + P],
                                         qtile[h * 64:h * 64 + 64, :],
                                         start=True, stop=True)
                    ptw = pts.tile([P, 2 * QC], F16, tag="ptw", name="ptw")
                    nc.scalar.activation(ptw[:], sw[:], Exp, scale=0.125)
                    pv_in = {h: ptw[:, h * QC:(h + 1) * QC] for h in range(HPC)}
                    c0 = 0
                else:
                    # diagonal k-tile: h0 scores land at [c0:QC], h1 at
                    # [QC:2*QC-c0] (shifted left so one exp covers both)
                    c0 = P * tdiag
                    ptag = f"ptd{tdiag}"
                    pt = ptd.tile([P, 2 * QC], F16, tag=ptag, name=ptag)
                    nc.tensor.matmul(sw[:, c0:QC],
                                     ktile[0:64, ko:ko + P],
                                     qtile[0:64, c0:QC], start=True, stop=True)
                    nc.tensor.matmul(sw[:, QC:2 * QC - c0],
                                     ktile[64:128, ko:ko + P],
                                     qtile[64:128, c0:QC], start=True, stop=True)
                    if c0 > 0 and ptag not in diag_zeroed:
                        nc.vector.memset(pt[:, 0:c0], 0.0)
                        diag_zeroed.add(ptag)
                    nc.scalar.activation(pt[:, c0:2 * QC - c0],
                                         sw[:, c0:2 * QC - c0], Exp, scale=0.125)
                    nc.vector.tensor_mul(pt[:, c0:c0 + P], pt[:, c0:c0 + P],
                                         tri_sb[:])
                    nc.vector.tensor_mul(pt[:, QC:QC + P], pt[:, QC:QC + P],
                                         tri_sb[:])
                    pv_in = {0: pt[:, c0:QC], 1: pt[:, QC:2 * QC - c0]}

                def emit_pv(kt=kt, pv_in=pv_in, c0=c0, bb=bb, pc=pc,
                            nkt_i=nkt_i):
                    for h in range(HPC):
                        vt = v_tiles[(bb, kt, h)]
                        nc.tensor.matmul(pc[h][0:65, c0:QC], vt[:], pv_in[h],
                                         start=(kt == 0),
                                         stop=(kt == nkt_i - 1))
                return emit_pv

            pv_prev = emit_qk_exp(0)
            for kt in range(1, nkt_i):
                pv_next = emit_qk_exp(kt)
                pv_prev()
                pv_prev = pv_next
            pv_prev()

            def norm_wo(bb=bb, i=i, j=j, pc=pc, ctx_t=ctx_t):
                # normalize: ctx^T[h] *= 1/sums[h] (partition-broadcast via a
                # K=1 PE matmul with a ones row)
                # normalize: ctx[h] rows 0:64 scaled by 1/sums (row 64).
                # Broadcast 1/sums across partitions with a K=1 fp16 matmul;
                # ACT moves it to SBUF so the multiply reads only one PSUM
                # operand. h1's result is lifted to partitions 64:128 of
                # ctx_t with a small SBUF->SBUF DMA (cross-partition moves
                # are DMA-only).
                for h in range(HPC):
                    rc32 = rcs.tile([P, QC], F32, tag="rc32", name="rc32")
                    nc.vector.reciprocal(rc32[64:65, :], pc[h][64:65, :])
                    rc16 = rcs.tile([P, QC], F16, tag="rc16", name="rc16")
                    nc.vector.tensor_copy(rc16[64:65, :], rc32[64:65, :])
                    bc = ps_tile("bk6")
                    bc_sb = rcs.tile([P, QC], F16, tag="bcsb", name="bcsb")
                    nc.tensor.matmul(bc[0:64, :], ones_sb[64:65, :],
                                     rc16[64:65, :], start=True, stop=True)
                    nc.scalar.copy(bc_sb[0:64, :], bc[0:64, :])
                    if h == 0:
                        nc.vector.tensor_mul(ctx_t[0:64, :], pc[0][0:64, :],
                                             bc_sb[0:64, :])
                    else:
                        ctxh1 = rcs.tile([P, QC], F16, tag="ctxh1",
                                         name="ctxh1")
                        nc.vector.tensor_mul(ctxh1[0:64, :], pc[1][0:64, :],
                                             bc_sb[0:64, :])
                        nc.sync.dma_start(ctx_t[64:128, :], ctxh1[0:64, :])

                # output projection for this chunk (bank bk6; bk7 stays free
                # for the next chunk's projections)
                last_chunk = (bb == b - 1 and i == nch - 1)
                for m in range(nf):
                    po = ps_tile("bk7" if (last_chunk and m % 2) else "bk6")
                    nc.tensor.matmul(po[:], wo_sb[:, m * P:(m + 1) * P], ctx_t[:],
                                     start=True, stop=True)
                    ot = wout.tile([P, QC], F16, tag="wo_out", name="wo_out")
                    nc.vector.tensor_copy(ot[:], po[:])
                    nc.gpsimd.dma_start(
                        out_t[m * P:(m + 1) * P,
                              bb * s + i * QC: bb * s + (i + 1) * QC],
                        ot[:])

            pending_norm_wo = norm_wo

    pending_norm_wo()


def build_program(s=S, d=D, b=B):
    import concourse.tile as tile
    from concourse import bacc, mybir
    from contextlib import ExitStack

    F16 = mybir.dt.float16
    F32 = mybir.dt.float32
    bs = b * s

    nc = bacc.Bacc("TRN2", target_bir_lowering=False, debug=False)
    io = {
        "xq_t": nc.dram_tensor("xq_t", [d, bs], F16, kind="ExternalInput").ap(),
        "xk_t": nc.dram_tensor("xk_t", [d, bs], F16, kind="ExternalInput").ap(),
        "xv_t": nc.dram_tensor("xv_t", [d, bs], F16, kind="ExternalInput").ap(),
        "wq": nc.dram_tensor("wq", [P, d], F16, kind="ExternalInput").ap(),
        "wk": nc.dram_tensor("wk", [P, d], F16, kind="ExternalInput").ap(),
        "wv": nc.dram_tensor("wv", [P, d], F16, kind="ExternalInput").ap(),
        "wo": nc.dram_tensor("wo", [P, d], F16, kind="ExternalInput").ap(),
        "bq": nc.dram_tensor("bq", [P, 1], F32, kind="ExternalInput").ap(),
        "bk": nc.dram_tensor("bk", [P, 1], F32, kind="ExternalInput").ap(),
        "tri": nc.dram_tensor("tri", [P, P], F16, kind="ExternalInput").ap(),
        "out_t": nc.dram_tensor("out_t", [d, bs], F16, kind="ExternalOutput").ap(),
    }
    with tile.TileContext(nc) as tc, ExitStack() as ctx:
        _mha_body(ctx, tc, io, s, d, b)
    nc.compile()
    return nc


# ---------------------------------------------------------------------------
# Host side
# ---------------------------------------------------------------------------

def _np_reference(query, key, value, mask, Wq, bq, Wk, bk, Wv, bv, Wo, bo):
    """Pure-numpy fallback, exact reference math (used only if the mask is
    not the expected causal mask)."""
    q = (query.reshape(-1, D) @ Wq + bq).reshape(B, S, H, DK).transpose(0, 2, 1, 3)
    k = (key.reshape(-1, D) @ Wk + bk).reshape(B, S, H, DK).transpose(0, 2, 1, 3)
    v = (value.reshape(-1, D) @ Wv + bv).reshape(B, S, H, DK).transpose(0, 2, 1, 3)
    scores = np.einsum("bhqd,bhkd->bhqk", q, k) / math.sqrt(DK)
    scores = np.where(mask[:, None, :, :] == 0, np.float32(-1e9), scores)
    scores -= scores.max(axis=-1, keepdims=True)
    p = np.exp(scores)
    p /= p.sum(axis=-1, keepdims=True)
    x = np.einsum("bhqk,bhkd->bhqd", p, v)
    x = x.transpose(0, 2, 1, 3).reshape(B, -1, D)
    return (x @ Wo + bo).astype(np.float32)


def _wlayout(w):
    """[D, 128] weight slice -> [128, (D//128)*128] fp16, partition-major:
    out[p, o*128 + m] = w[o*128 + p, m] (contiguous 256-element rows per
    partition => efficient DMA)."""
    d = w.shape[0]
    nf = d // P
    return np.ascontiguousarray(
        w.reshape(nf, P, P).transpose(1, 0, 2).reshape(P, nf * P)).astype(np.float16)


def _shard_inputs(query, key, value, Wq, bq, Wk, bk, Wv, Wo):
    f16 = np.float16
    xq_t = np.ascontiguousarray(query.reshape(B * S, D).T).astype(f16)
    xk_t = np.ascontiguousarray(key.reshape(B * S, D).T).astype(f16)
    xv_t = np.ascontiguousarray(value.reshape(B * S, D).T).astype(f16)
    idx = np.arange(P)
    tri = (idx[:, None] <= idx[None, :]).astype(f16)  # tri[k, q] = k <= q
    in_maps = []
    for c in range(NCORES):
        sl = slice(c * HPC * DK, (c + 1) * HPC * DK)
        in_maps.append({
            "xq_t": xq_t,
            "xk_t": xk_t,
            "xv_t": xv_t,
            "wq": _wlayout(Wq[:, sl]),
            "wk": _wlayout(Wk[:, sl]),
            "wv": _wlayout(Wv[:, sl]),
            "wo": np.ascontiguousarray(Wo[sl, :]).astype(f16),
            "bq": np.ascontiguousarray(bq[sl]).reshape(P, 1).astype(np.float32),
            "bk": np.ascontiguousarray(bk[sl]).reshape(P, 1).astype(np.float32),
            "tri": tri,
        })
    return in_maps


def kernel(**inputs):
    query = np.asarray(inputs["query"], np.float32)
    key = np.asarray(inputs["key"], np.float32)
    value = np.asarray(inputs["value"], np.float32)
    mask = np.asarray(inputs["mask"])
    Wq = np.asarray(inputs["Wq"], np.float32)
    bq = np.asarray(inputs["bq"], np.float32)
    Wk = np.asarray(inputs["Wk"], np.float32)
    bk = np.asarray(inputs["bk"], np.float32)
    Wv = np.asarray(inputs["Wv"], np.float32)
    bv = np.asarray(inputs["bv"], np.float32)
    Wo = np.asarray(inputs["Wo"], np.float32)
    bo = np.asarray(inputs["bo"], np.float32)

    # The device program hardcodes causal structure; verify and fall back
    # to exact host math for any other mask.
    tril = np.tril(np.ones((S, S), np.int8))
    if mask.shape != (B, S, S) or not np.array_equal(
            (mask != 0).astype(np.int8), np.broadcast_to(tril, (B, S, S))):
        return _np_reference(query, key, value, mask,
                             Wq, bq, Wk, bk, Wv, bv, Wo, bo)

    in_maps = _shard_inputs(query, key, value, Wq, bq, Wk, bk, Wv, Wo)
    outs = _run_spmd(in_maps)

    acc = outs.astype(np.float32).sum(axis=0)  # [D, B*S]
    out = acc.T + (bv @ Wo + bo)[None, :]
    return out.reshape(B, S, D).astype(np.float32)


def _get_exec():
    """Build (once) the program + jitted SPMD executable."""
    if "exec" in _PROGRAM_CACHE:
        return _PROGRAM_CACHE["exec"]
    import jax
    from jax.sharding import Mesh, PartitionSpec
    from jax.experimental.shard_map import shard_map
    import concourse.mybir as mybir
    from concourse import bass2jax

    nc = build_program()
    _PROGRAM_CACHE["nc"] = nc
    bass2jax.install_neuronx_cc_hook()
    partition_name = nc.partition_id_tensor.name if nc.partition_id_tensor else None
    in_names, out_names, out_avals, zero_outs = [], [], [], []
    for alloc in nc.m.functions[0].allocations:
        if not isinstance(alloc, mybir.MemoryLocationSet):
            continue
        name = alloc.memorylocations[0].name
        if alloc.kind == "ExternalInput":
            if name != partition_name:
                in_names.append(name)
        elif alloc.kind == "ExternalOutput":
            out_names.append(name)
            shape = tuple(alloc.tensor_shape)
            dtype = mybir.dt.np(alloc.dtype)
            out_avals.append(jax.core.ShapedArray(shape, dtype))
            zero_outs.append(np.zeros(shape, dtype))
    n_params = len(in_names)
    all_in_names = list(in_names) + list(out_names)
    if partition_name is not None:
        all_in_names.append(partition_name)

    def _body(*args):
        operands = list(args)
        if partition_name is not None:
            operands.append(bass2jax.partition_id_tensor())
        return tuple(bass2jax._bass_exec_p.bind(
            *operands,
            out_avals=tuple(out_avals),
            in_names=tuple(all_in_names),
            out_names=tuple(out_names),
            lowering_input_output_aliases=(),
            sim_require_finite=True,
            sim_require_nnan=True,
            nc=nc,
        ))

    devices = jax.devices()[:NCORES]
    assert len(devices) >= NCORES, f"need {NCORES} neuron cores, have {len(devices)}"
    mesh = Mesh(np.asarray(devices[:NCORES]), ("core",))
    fn = jax.jit(
        shard_map(_body, mesh=mesh,
                  in_specs=(PartitionSpec("core"),) * (n_params + len(zero_outs)),
                  out_specs=(PartitionSpec("core"),) * len(out_names),
                  check_rep=False),
        donate_argnums=tuple(range(n_params, n_params + len(out_names))),
        keep_unused=True)
    _PROGRAM_CACHE["exec"] = (fn, in_names, zero_outs)
    return _PROGRAM_CACHE["exec"]


def _run_spmd(in_maps):
    """Run the SPMD program on 8 cores; returns per-core out_t [8, D, B*S]."""
    fn, in_names, zero_outs = _get_exec()
    concat_in = [np.concatenate([np.asarray(in_maps[c][nm])
                                 for c in range(NCORES)], axis=0)
                 for nm in in_names]
    concat_zero = [np.zeros((NCORES * z.shape[0], *z.shape[1:]), z.dtype)
                   for z in zero_outs]
    out = fn(*concat_in, *concat_zero)
    LAST["out"] = out
    return np.asarray(out[0]).reshape(NCORES, D, B * S)



# revision 2
# speedup vs baseline: 1.4355x; 1.4355x over previous
"""Trainium2 Bass kernel for 16-head causal multi-head attention (v2).

Problem: B=2, S=2048, D=1024, H=16 (head dim 64), causal mask.
    out = softmax((XqWq+bq)(XkWk+bk)^T / 8, causal) (XvWv+bv) Wo + bo

Sharding: 8 cores = 2 batches x 4 head-groups. Core c owns batch c//4 and
heads 4*(c%4)..4*(c%4)+3 (dk dims 256): Wq/Wk/Wv column-sliced, Wo
row-sliced. Each core computes a partial y^T [1024, 2048] for its batch;
host sums the 4 partials per batch and adds (bv @ Wo + bo).

Device-side design (per core):
  - Projections in fp8e4 DoubleRow with residual hi/lo split:
    q = X_hi@W_hi + (X_lo@W_hi + X_hi@W_lo), W pre-scaled by 32 so its fp8
    values are unit-variance; the 1/32 is folded into the PSUM eviction.
    Cost model charge: 0.75x of fp16, at ~2e-3 end-to-end error.
  - QK in fp16, scores transposed S^T[k, q] so exp output P^T is ready as
    the PV stationary operand.
  - PV "swapped": out ctx[q, dk+1] with P^T [k, q-subtile] stationary and
    V-augmented [k, 65] moving -> charge 65 rows per (ktile, qsub, head)
    instead of 512 (2x PE saving); the ones column (x32) yields softmax
    sums so normalization is a per-partition (per-token) scalar multiply.
  - ctx is transposed back (PE is_transpose matmuls into one psum bank)
    for the fp16 Wo matmuls; Wo/transpose work of chunk j is emitted as
    PE filler inside chunk j+1's ACT-bound attention loop.
  - 4 heads are processed as 2 passes of 2 heads to fit PSUM:
    banks = scores dbuf (2x2) + PV accum (2) + aux dbuf (2).
"""

import math

import numpy as np
import ml_dtypes

# Full-problem constants
B, S, D, H = 2, 2048, 1024, 16
DK = D // H  # 64
NCORES = 8
HPC = 4          # heads per core
DKC = HPC * DK   # 256 dk dims per core
P = 128
QC = 512         # tokens per chunk
NCH = S // QC    # 4 chunks
KPC = QC // P    # 4 k-tiles per chunk
NF = D // P      # 8 feature tiles
WSCALE = 32.0    # fp8 weight pre-scale

SPLIT_PROJ = True  # fp8 DoubleRow split projections (False -> fp16 proj)
DEBUG = False      # add debug dram dumps of intermediates

_PROGRAM_CACHE = {}
LAST = {}

F8NP = ml_dtypes.float8_e4m3


# ---------------------------------------------------------------------------
# Device program
# ---------------------------------------------------------------------------

def _mha_body(ctx, tc, io):
    from concourse import mybir

    F8 = mybir.dt.float8e4
    F16 = mybir.dt.float16
    F32 = mybir.dt.float32
    Exp = mybir.ActivationFunctionType.Exp
    DR = mybir.MatmulPerfMode.DoubleRow
    MUL = mybir.AluOpType.mult
    ADD = mybir.AluOpType.add

    nc = tc.nc

    consts = ctx.enter_context(tc.tile_pool(name="consts", bufs=1))
    xs = ctx.enter_context(tc.tile_pool(name="xs", bufs=1))
    persist = ctx.enter_context(tc.tile_pool(name="persist", bufs=1))
    pts = ctx.enter_context(tc.tile_pool(name="pts", bufs=4))
    wouts = ctx.enter_context(tc.tile_pool(name="wouts", bufs=4))
    pspool = ctx.enter_context(tc.tile_pool(name="ps", bufs=1, space="PSUM"))

    # ---- constants & inputs (DMA order = first-use order) ----------------
    # chunk-0 prologue data first (wq/wk/wv + first-chunk x), tiny biases,
    # then the rest of x, then late-use constants (tri/eye/wo).
    if SPLIT_PROJ:
        wdt, wshape = F8, [P, NF, 2, DKC]
    else:
        wdt, wshape = F16, [P, NF, 1, DKC]
    xdt = F8 if SPLIT_PROJ else F16
    nsel = 2 if SPLIT_PROJ else 1

    bx0, bxr = {}, {}
    bq_sb = consts.tile([P, 2], F32, tag="bq", name="bq_sb")
    nc.sync.dma_start(bq_sb[:], io["bq"])
    bk_sb = consts.tile([P, 2], F32, tag="bk", name="bk_sb")
    nc.sync.dma_start(bk_sb[:], io["bk"])
    w_sbs = {}
    for wnm, xnm in (("wq", "q"), ("wk", "k"), ("wv", "v")):
        w_sbs[wnm] = consts.tile(wshape, wdt, tag=wnm, name=wnm + "_sb")
        nc.sync.dma_start(w_sbs[wnm][:], io[wnm])
        t0 = xs.tile([P, NF, nsel, QC], xdt, tag=f"x{xnm}0", name=f"x{xnm}0")
        nc.sync.dma_start(t0[:], io["x" + xnm][:, :, :, 0:QC])
        bx0[xnm] = t0
    wq_sb, wk_sb, wv_sb = w_sbs["wq"], w_sbs["wk"], w_sbs["wv"]
    tri_sb = consts.tile([P, P], F16, tag="tri", name="tri_sb")
    nc.sync.dma_start(tri_sb[:], io["tri"])
    eye_sb = consts.tile([P, P], F16, tag="eye", name="eye_sb")
    nc.sync.dma_start(eye_sb[:], io["eye"])
    for nm in ("q", "k", "v"):
        bxr[nm] = xs.tile([P, NF, nsel, S - QC], xdt, tag=f"x{nm}r",
                          name=f"x{nm}r")
    # per-chunk input DMAs, interleaved q/k/v, in consumption order
    for j in range(1, NCH):
        for nm in ("q", "k", "v"):
            nc.sync.dma_start(
                bxr[nm][:, :, :, (j - 1) * QC:j * QC],
                io["x" + nm][:, :, :, j * QC:(j + 1) * QC])
    wo_sb = consts.tile([P, 2, NF, P], F16, tag="wo", name="wo_sb")
    nc.sync.dma_start(wo_sb[:], io["wo"])

    def xsl(nm, lo, hi):
        """[P, NF, nsel, hi-lo] view of input nm tokens [lo:hi)."""
        if hi <= QC:
            return bx0[nm][:, :, :, lo:hi]
        return bxr[nm][:, :, :, lo - QC:hi - QC]

    def ps_tile(tag, shape=None, dtype=F32, bufs=None):
        shape = shape or [P, QC]
        return pspool.tile(shape, dtype, tag=tag, name=tag, bufs=bufs)

    # ---- projection units (queued as PE fillers) -------------------------
    def mk_proj_qk_unit(j, w_sb, b_sb, out, t, srcnm):
        """One psum tile of a Q/K projection: matmuls + biased fp16 evict."""
        co = j * QC

        def go():
            pp = ps_tile("aux", bufs=2)
            if SPLIT_PROJ:
                first = True
                for hh in range(2):  # token halves (moving free <= 512)
                    ts0, ts1 = co + hh * 256, co + (hh + 1) * 256
                    for f in range(0, NF, 2):  # main terms: hi x hi pairs
                        nc.tensor.matmul(
                            pp[:, hh * 256:(hh + 1) * 256],
                            w_sb[:, f:f + 2, 0, t * P:(t + 1) * P],
                            xsl(srcnm, ts0, ts1)[:, f:f + 2, 1, :],
                            start=first, stop=False, perf_mode=DR,
                            skip_group_check=True)
                        first = False
                    for f in range(NF):  # cross terms: (hi,lo)x(lo,hi)
                        nc.tensor.matmul(
                            pp[:, hh * 256:(hh + 1) * 256],
                            w_sb[:, f, 0:2, t * P:(t + 1) * P],
                            xsl(srcnm, ts0, ts1)[:, f, 0:2, :],
                            start=False, stop=(f == NF - 1 and hh == 1),
                            perf_mode=DR, skip_group_check=True)
                scl = 1.0 / WSCALE
            else:
                for f in range(NF):
                    nc.tensor.matmul(
                        pp[:], w_sb[:, f, 0, t * P:(t + 1) * P],
                        xsl(srcnm, co, co + QC)[:, f, 0, :],
                        start=(f == 0), stop=(f == NF - 1))
                scl = 1.0
            nc.vector.tensor_scalar(out[:, t, :], pp[:], scl,
                                    b_sb[:, t:t + 1], MUL, ADD)
        return go

    def mk_proj_v_unit(j, t4):
        """One 128-token tile of the V projection -> vt tile (token-major)."""
        co = j * QC

        def go():
            pp = ps_tile("aux", [P, DKC], bufs=2)
            ts0, ts1 = co + t4 * P, co + (t4 + 1) * P
            if SPLIT_PROJ:
                first = True
                for f in range(0, NF, 2):
                    nc.tensor.matmul(
                        pp[:], xsl("v", ts0, ts1)[:, f:f + 2, 1, :],
                        wv_sb[:, f:f + 2, 0, :],
                        start=first, stop=False, perf_mode=DR,
                        skip_group_check=True)
                    first = False
                for f in range(NF):
                    nc.tensor.matmul(
                        pp[:], xsl("v", ts0, ts1)[:, f, 0:2, :],
                        wv_sb[:, f, 0:2, :],
                        start=False, stop=(f == NF - 1),
                        perf_mode=DR, skip_group_check=True)
                scl = 1.0 / WSCALE
            else:
                for f in range(NF):
                    nc.tensor.matmul(
                        pp[:], xsl("v", ts0, ts1)[:, f, 0, :],
                        wv_sb[:, f, 0, :],
                        start=(f == 0), stop=(f == NF - 1))
                scl = 1.0
            kt = j * KPC + t4
            vt = vt_tiles[kt]
            # ones column: PV's column 64 accumulates the softmax sums
            nc.vector.memset(vt[:, :, 64:65], 1.0)
            nc.vector.tensor_scalar_mul(
                vt[:, :, 0:64], pp[:].rearrange("p (h d) -> p h d", d=64), scl)
        return go

    qt = {}
    kt_tiles = {}
    vt_tiles = {}

    # Filler queues of PE closures drained inside ACT-bound attention
    # k-loops: `proj_fillers` (next chunk's projections + ctx transposes —
    # must drain promptly) and `wo_fillers` (Wo matmuls + output DMA of
    # finished chunks — deferred into the LAST chunk's long k-loop, which
    # is the only ACT-bound phase with spare PE time).
    proj_fillers = []
    wo_fillers = []

    def drain_filler(n=1, wo_ok=False):
        for _ in range(n):
            if proj_fillers:
                proj_fillers.pop(0)()
            elif wo_ok and wo_fillers:
                wo_fillers.pop(0)()

    def queue_proj_qk(jn):
        for (w_sb, b_sb, tag, srcnm) in ((wq_sb, bq_sb, f"qt{jn % 2}", "q"),
                                         (wk_sb, bk_sb, f"kt{jn}", "k")):
            out = persist.tile([P, 2, QC], F16, tag=tag, name=tag)
            if srcnm == "q":
                qt[jn % 2] = out
            else:
                kt_tiles[jn] = out
            for t in range(2):
                proj_fillers.append(
                    mk_proj_qk_unit(jn, w_sb, b_sb, out, t, srcnm))

    def queue_proj_v(jn):
        for t4 in range(KPC):
            kt = jn * KPC + t4
            vt_tiles[kt] = persist.tile([P, HPC, 65], F16, tag=f"vt{kt}",
                                        name=f"vt{kt}")
            proj_fillers.append(mk_proj_v_unit(jn, t4))

    def queue_proj(jn):
        queue_proj_qk(jn)
        queue_proj_v(jn)

    # ---- attention pass (2 heads) ---------------------------------------
    def emit_pass(j, p):
        """Attention for chunk j, heads {2p, 2p+1}. Returns pvacc tiles."""
        nkt = KPC * (j + 1)
        qtile = qt[j % 2]
        pv = [ps_tile(f"pv{l}", [P, KPC, 80]) for l in range(2)]

        def head_slices(l):
            """(dim-tile t, partition offset) for local head l of pass p."""
            hh = 2 * p + l
            return hh // 2, (hh % 2) * 64

        def emit_qk_exp(kt):
            jk = kt // KPC          # source chunk of this k-tile
            ko = (kt % KPC) * P
            tdiag = kt - KPC * j    # >=0 on the diagonal chunk
            ktile = kt_tiles[jk]
            sw = ps_tile("swA" if kt % 2 == 0 else "swB", [P, 2 * QC])
            c0 = max(tdiag, 0) * P
            for l in range(2):
                t, po = head_slices(l)
                nc.tensor.matmul(
                    sw[:, l * QC + c0:(l + 1) * QC],
                    ktile[po:po + 64, t, ko:ko + P],
                    qtile[po:po + 64, t, c0:QC],
                    start=True, stop=True)
            pt = pts.tile([P, 2 * QC], F16, tag="pt", name="pt")
            if c0:
                # one strided exp covering both heads' live columns
                swv = sw[:].rearrange("p (l q) -> p l q", l=2)[:, :, c0:QC]
                ptv = pt[:].rearrange("p (l q) -> p l q", l=2)[:, :, c0:QC]
                nc.scalar.activation(ptv, swv, Exp, scale=0.125)
            else:
                nc.scalar.activation(pt[:], sw[:], Exp, scale=0.125)
            if tdiag >= 0:
                # masked-diagonal multiply on the (otherwise idle) Pool
                # engine so exp-dependent work never blocks DVE's evictions
                for l in range(2):
                    nc.gpsimd.tensor_mul(pt[:, l * QC + c0:l * QC + c0 + P],
                                         pt[:, l * QC + c0:l * QC + c0 + P],
                                         tri_sb[:])
            if DEBUG and j == 0 and p == 0 and kt == 0:
                nc.sync.dma_start(io["dbg_pt"], pt[:])

            def emit_pv():
                vt = vt_tiles[kt]
                for l in range(2):
                    hh = 2 * p + l
                    for t in range(max(tdiag, 0), KPC):
                        nc.tensor.matmul(
                            pv[l][:, t, 0:65],
                            pt[:, l * QC + t * P:l * QC + (t + 1) * P],
                            vt[:, hh, :],
                            start=(kt == 0 and t == 0),
                            stop=(kt == KPC * j + t),
                            skip_group_check=True)
            return emit_pv

        wo_ok = (j >= NCH - 2)
        pv_prev = emit_qk_exp(0)
        for kt in range(1, nkt):
            pv_next = emit_qk_exp(kt)
            drain_filler(2 if len(proj_fillers) > 8 else 1, wo_ok)
            pv_prev()
            pv_prev = pv_next
        drain_filler(1, wo_ok)
        pv_prev()
        return pv

    # ---- normalize + transpose + Wo for one chunk ------------------------
    def emit_norm(j, p, pv):
        """Normalize pass p of chunk j into ctx_sb (DVE), return ctx_sb."""
        cs = persist.tile([P, 2, KPC, 64], F16, tag=f"ctxs{p}",
                          name=f"ctxs{p}")
        for l in range(2):
            rc = persist.tile([P, KPC], F32, tag=f"rc{p}{l}", name=f"rc{p}{l}")
            nc.vector.reciprocal(rc[:], pv[l][:, :, 64:65].squeeze(2))
            for t in range(KPC):
                nc.vector.tensor_scalar_mul(cs[:, l, t, :], pv[l][:, t, 0:64],
                                            rc[:, t:t + 1])
        return cs

    def mk_transp(cs, ctx_t, p):
        def go():
            trp = ps_tile("aux", [P, QC], F16, bufs=2)
            for l in range(2):
                for t in range(KPC):
                    nc.tensor.matmul(
                        trp[l * 64:(l + 1) * 64, t * P:(t + 1) * P],
                        cs[:, l, t, :], eye_sb[:],
                        is_transpose=True,
                        start=(t == 0), stop=(t == KPC - 1),
                        skip_group_check=True,
                        tile_position=(0, l * 64))
            nc.vector.tensor_copy(ctx_t[:, p, :], trp[:])
        return go

    def mk_wo(ctx_t, wout, m, act_ok):
        def go():
            po = ps_tile("aux", [P, QC], bufs=2)
            for t in range(2):
                nc.tensor.matmul(po[:], wo_sb[:, t, m, :], ctx_t[:, t, :],
                                 start=(t == 0), stop=(t == 1))
            if act_ok and m % 2:
                # tail chunk: ACT is idle there, share the eviction load
                nc.scalar.copy(wout[:, m, :], po[:])
            else:
                nc.vector.tensor_copy(wout[:, m, :], po[:])
        return go

    def mk_out_dma(j, wout):
        def go():
            nc.gpsimd.dma_start(
                io["out_t"].rearrange("(m p) s -> p m s", p=P)
                [:, :, j * QC:(j + 1) * QC], wout[:])
        return go

    # ---- main loop -------------------------------------------------------
    # chunk 0's projections run up front; chunk j+1's are drained as PE
    # fillers inside chunk j's ACT-bound attention, together with chunk
    # j-1/j's transpose + Wo + output-DMA epilogue.
    queue_proj_qk(0)
    while proj_fillers:
        proj_fillers.pop(0)()
    queue_proj_v(0)
    for j in range(NCH):
        if j + 1 < NCH:
            queue_proj(j + 1)
        if DEBUG and j == 0:
            nc.sync.dma_start(io["dbg_qt"], qt[0][:])
            nc.sync.dma_start(io["dbg_kt"], kt_tiles[0][:])
            nc.sync.dma_start(io["dbg_vt"], vt_tiles[0][:])
        ctx_t = persist.tile([P, 2, QC], F16, tag=f"ctxt{j}",
                             name=f"ctxt{j}")
        wout = wouts.tile([P, NF, QC], F16, tag="wout", name="wout")
        css = []
        for p in range(2):
            pv = emit_pass(j, p)
            cs = emit_norm(j, p, pv)
            css.append(cs)
            proj_fillers.append(mk_transp(cs, ctx_t, p))
        if DEBUG and j == 0:
            nc.sync.dma_start(io["dbg_ctxs"], css[0][:])
        for m in range(NF):
            wo_fillers.append(mk_wo(ctx_t, wout, m, j == NCH - 1))
        wo_fillers.append(mk_out_dma(j, wout))
        # chunk j+1's attention reads qt/kt/vt(j+1): those projections must
        # be fully emitted before the next emit_pass
        while proj_fillers:
            proj_fillers.pop(0)()
    while wo_fillers:
        wo_fillers.pop(0)()


def build_program():
    import concourse.tile as tile
    from concourse import bacc, mybir
    from contextlib import ExitStack

    F8 = mybir.dt.float8e4
    F16 = mybir.dt.float16
    F32 = mybir.dt.float32

    nc = bacc.Bacc("TRN2", target_bir_lowering=False, debug=False)
    xdt = F8 if SPLIT_PROJ else F16
    nsel = 2 if SPLIT_PROJ else 1
    wdt = F8 if SPLIT_PROJ else F16
    io = {
        "xq": nc.dram_tensor("xq", [P, NF, nsel, S], xdt,
                             kind="ExternalInput").ap(),
        "xk": nc.dram_tensor("xk", [P, NF, nsel, S], xdt,
                             kind="ExternalInput").ap(),
        "xv": nc.dram_tensor("xv", [P, NF, nsel, S], xdt,
                             kind="ExternalInput").ap(),
        "wq": nc.dram_tensor("wq", [P, NF, nsel, DKC], wdt,
                             kind="ExternalInput").ap(),
        "wk": nc.dram_tensor("wk", [P, NF, nsel, DKC], wdt,
                             kind="ExternalInput").ap(),
        "wv": nc.dram_tensor("wv", [P, NF, nsel, DKC], wdt,
                             kind="ExternalInput").ap(),
        "wo": nc.dram_tensor("wo", [P, 2, NF, P], F16,
                             kind="ExternalInput").ap(),
        "tri": nc.dram_tensor("tri", [P, P], F16, kind="ExternalInput").ap(),
        "eye": nc.dram_tensor("eye", [P, P], F16, kind="ExternalInput").ap(),
        "bq": nc.dram_tensor("bq", [P, 2], F32, kind="ExternalInput").ap(),
        "bk": nc.dram_tensor("bk", [P, 2], F32, kind="ExternalInput").ap(),
        "out_t": nc.dram_tensor("out_t", [D, S], F16,
                                kind="ExternalOutput").ap(),
    }
    if DEBUG:
        io["dbg_qt"] = nc.dram_tensor("dbg_qt", [P, 2, QC], F16,
                                      kind="ExternalOutput").ap()
        io["dbg_kt"] = nc.dram_tensor("dbg_kt", [P, 2, QC], F16,
                                      kind="ExternalOutput").ap()
        io["dbg_vt"] = nc.dram_tensor("dbg_vt", [P, HPC, 65], F16,
                                      kind="ExternalOutput").ap()
        io["dbg_pt"] = nc.dram_tensor("dbg_pt", [P, 2 * QC], F16,
                                      kind="ExternalOutput").ap()
        io["dbg_ctxs"] = nc.dram_tensor("dbg_ctxs", [P, 2, KPC, 64], F16,
                                        kind="ExternalOutput").ap()
    with tile.TileContext(nc) as tc, ExitStack() as ctx:
        _mha_body(ctx, tc, io)
    nc.compile()
    return nc


# ---------------------------------------------------------------------------
# Host side
# ---------------------------------------------------------------------------

def _np_reference(query, key, value, mask, Wq, bq, Wk, bk, Wv, bv, Wo, bo):
    q = (query.reshape(-1, D) @ Wq + bq).reshape(B, S, H, DK).transpose(0, 2, 1, 3)
    k = (key.reshape(-1, D) @ Wk + bk).reshape(B, S, H, DK).transpose(0, 2, 1, 3)
    v = (value.reshape(-1, D) @ Wv + bv).reshape(B, S, H, DK).transpose(0, 2, 1, 3)
    scores = np.einsum("bhqd,bhkd->bhqk", q, k) / math.sqrt(DK)
    scores = np.where(mask[:, None, :, :] == 0, np.float32(-1e9), scores)
    scores -= scores.max(axis=-1, keepdims=True)
    p = np.exp(scores)
    p /= p.sum(axis=-1, keepdims=True)
    x = np.einsum("bhqk,bhkd->bhqd", p, v)
    x = x.transpose(0, 2, 1, 3).reshape(B, -1, D)
    return (x @ Wo + bo).astype(np.float32)


def _split8(x):
    """fp32 -> (lo, hi) fp8e4 with hi = fp8(x), lo = fp8(x - hi)."""
    hi = x.astype(F8NP)
    lo = (x - hi.astype(np.float32)).astype(F8NP)
    return lo, hi


def _xlayout(x):
    """[S, D] batch slice -> [128, NF, nsel, S]: [p, f, sel, t] with
    sel = (lo, hi) fp8 split (or a single fp16 plane)."""
    xt = np.ascontiguousarray(x.T)  # [D, S]
    xt = xt.reshape(NF, P, S).transpose(1, 0, 2)  # [p, f, t]
    if not SPLIT_PROJ:
        return np.ascontiguousarray(xt[:, :, None, :]).astype(np.float16)
    lo, hi = _split8(xt.astype(np.float32))
    out = np.empty((P, NF, 2, S), F8NP)
    out[:, :, 0, :] = lo
    out[:, :, 1, :] = hi
    return out


def _wlayout(w):
    """[D, DKC] weight slice -> [128, NF, nsel, DKC]: [p, f, sel, m] with
    sel = (hi, lo) of the x32-scaled weight (or single fp16 plane)."""
    wr = w.reshape(NF, P, DKC).transpose(1, 0, 2)  # [p, f, m]
    if not SPLIT_PROJ:
        return np.ascontiguousarray(wr[:, :, None, :]).astype(np.float16)
    lo, hi = _split8(wr.astype(np.float32) * WSCALE)
    out = np.empty((P, NF, 2, DKC), F8NP)
    out[:, :, 0, :] = hi
    out[:, :, 1, :] = lo
    return out


def _shard_inputs(query, key, value, Wq, bq, Wk, bk, Wv, Wo):
    idx = np.arange(P)
    tri = (idx[:, None] <= idx[None, :]).astype(np.float16)  # [k, q] k<=q
    eye = np.eye(P, dtype=np.float16)
    in_maps = []
    xcache = {}
    for c in range(NCORES):
        bb, g = c // 4, c % 4
        sl = slice(g * DKC, (g + 1) * DKC)
        if bb not in xcache:
            xcache[bb] = {
                "xq": _xlayout(query[bb]),
                "xk": _xlayout(key[bb]),
                "xv": _xlayout(value[bb]),
            }
        wo = Wo[sl, :].reshape(2, P, NF, P).transpose(1, 0, 2, 3)
        in_maps.append({
            **xcache[bb],
            "wq": _wlayout(Wq[:, sl]),
            "wk": _wlayout(Wk[:, sl]),
            "wv": _wlayout(Wv[:, sl]),
            "wo": np.ascontiguousarray(wo).astype(np.float16),
            "tri": tri,
            "eye": eye,
            "bq": np.ascontiguousarray(bq[sl].reshape(2, P).T).astype(np.float32),
            "bk": np.ascontiguousarray(bk[sl].reshape(2, P).T).astype(np.float32),
        })
    return in_maps


def kernel(**inputs):
    query = np.asarray(inputs["query"], np.float32)
    key = np.asarray(inputs["key"], np.float32)
    value = np.asarray(inputs["value"], np.float32)
    mask = np.asarray(inputs["mask"])
    Wq = np.asarray(inputs["Wq"], np.float32)
    bq = np.asarray(inputs["bq"], np.float32)
    Wk = np.asarray(inputs["Wk"], np.float32)
    bk = np.asarray(inputs["bk"], np.float32)
    Wv = np.asarray(inputs["Wv"], np.float32)
    bv = np.asarray(inputs["bv"], np.float32)
    Wo = np.asarray(inputs["Wo"], np.float32)
    bo = np.asarray(inputs["bo"], np.float32)

    tril = np.tril(np.ones((S, S), np.int8))
    if mask.shape != (B, S, S) or not np.array_equal(
            (mask != 0).astype(np.int8), np.broadcast_to(tril, (B, S, S))):
        return _np_reference(query, key, value, mask,
                             Wq, bq, Wk, bk, Wv, bv, Wo, bo)

    in_maps = _shard_inputs(query, key, value, Wq, bq, Wk, bk, Wv, Wo)
    outs = _run_spmd(in_maps)  # [8, D, S]

    outs = outs.astype(np.float32)
    corr = (bv @ Wo + bo)[None, :]
    res = np.empty((B, S, D), np.float32)
    for bb in range(B):
        acc = outs[bb * 4:(bb + 1) * 4].sum(axis=0)  # [D, S]
        res[bb] = acc.T + corr
    return res


def _get_exec():
    if "exec" in _PROGRAM_CACHE:
        return _PROGRAM_CACHE["exec"]
    import jax
    from jax.sharding import Mesh, PartitionSpec
    from jax.experimental.shard_map import shard_map
    import concourse.mybir as mybir
    from concourse import bass2jax

    nc = build_program()
    _PROGRAM_CACHE["nc"] = nc
    bass2jax.install_neuronx_cc_hook()
    partition_name = nc.partition_id_tensor.name if nc.partition_id_tensor else None
    in_names, out_names, out_avals, zero_outs = [], [], [], []
    for alloc in nc.m.functions[0].allocations:
        if not isinstance(alloc, mybir.MemoryLocationSet):
            continue
        name = alloc.memorylocations[0].name
        if alloc.kind == "ExternalInput":
            if name != partition_name:
                in_names.append(name)
        elif alloc.kind == "ExternalOutput":
            out_names.append(name)
            shape = tuple(alloc.tensor_shape)
            dtype = mybir.dt.np(alloc.dtype)
            out_avals.append(jax.core.ShapedArray(shape, dtype))
            zero_outs.append(np.zeros(shape, dtype))
    n_params = len(in_names)
    all_in_names = list(in_names) + list(out_names)
    if partition_name is not None:
        all_in_names.append(partition_name)

    def _body(*args):
        operands = list(args)
        if partition_name is not None:
            operands.append(bass2jax.partition_id_tensor())
        return tuple(bass2jax._bass_exec_p.bind(
            *operands,
            out_avals=tuple(out_avals),
            in_names=tuple(all_in_names),
            out_names=tuple(out_names),
            lowering_input_output_aliases=(),
            sim_require_finite=True,
            sim_require_nnan=True,
            nc=nc,
        ))

    devices = jax.devices()[:NCORES]
    assert len(devices) >= NCORES, f"need {NCORES} cores, have {len(devices)}"
    mesh = Mesh(np.asarray(devices[:NCORES]), ("core",))
    fn = jax.jit(
        shard_map(_body, mesh=mesh,
                  in_specs=(PartitionSpec("core"),) * (n_params + len(zero_outs)),
                  out_specs=(PartitionSpec("core"),) * len(out_names),
                  check_rep=False),
        donate_argnums=tuple(range(n_params, n_params + len(out_names))),
        keep_unused=True)
    _PROGRAM_CACHE["exec"] = (fn, in_names, zero_outs)
    return _PROGRAM_CACHE["exec"]


def _run_spmd(in_maps):
    fn, in_names, zero_outs = _get_exec()
    concat_in = [np.concatenate([np.asarray(in_maps[c][nm])
                                 for c in range(NCORES)], axis=0)
                 for nm in in_names]
    concat_zero = [np.zeros((NCORES * z.shape[0], *z.shape[1:]), z.dtype)
                   for z in zero_outs]
    out = fn(*concat_in, *concat_zero)
    LAST["out"] = out
    return np.asarray(out[0]).reshape(NCORES, D, S)


# revision 3
# speedup vs baseline: 1.4458x; 1.0072x over previous
"""Trainium2 Bass kernel for 16-head causal multi-head attention (v2).

Problem: B=2, S=2048, D=1024, H=16 (head dim 64), causal mask.
    out = softmax((XqWq+bq)(XkWk+bk)^T / 8, causal) (XvWv+bv) Wo + bo

Sharding: 8 cores = 2 batches x 4 head-groups. Core c owns batch c//4 and
heads 4*(c%4)..4*(c%4)+3 (dk dims 256): Wq/Wk/Wv column-sliced, Wo
row-sliced. Each core computes a partial y^T [1024, 2048] for its batch;
host sums the 4 partials per batch and adds (bv @ Wo + bo).

Device-side design (per core):
  - Projections in fp8e4 DoubleRow with residual hi/lo split:
    q = X_hi@W_hi + (X_lo@W_hi + X_hi@W_lo), W pre-scaled by 32 so its fp8
    values are unit-variance; the 1/32 is folded into the PSUM eviction.
    Cost model charge: 0.75x of fp16, at ~2e-3 end-to-end error.
  - QK in fp16, scores transposed S^T[k, q] so exp output P^T is ready as
    the PV stationary operand.
  - PV "swapped": out ctx[q, dk+1] with P^T [k, q-subtile] stationary and
    V-augmented [k, 65] moving -> charge 65 rows per (ktile, qsub, head)
    instead of 512 (2x PE saving); the ones column (x32) yields softmax
    sums so normalization is a per-partition (per-token) scalar multiply.
  - ctx is transposed back (PE is_transpose matmuls into one psum bank)
    for the fp16 Wo matmuls; Wo/transpose work of chunk j is emitted as
    PE filler inside chunk j+1's ACT-bound attention loop.
  - 4 heads are processed as 2 passes of 2 heads to fit PSUM:
    banks = scores dbuf (2x2) + PV accum (2) + aux dbuf (2).
"""

import math

import numpy as np
import ml_dtypes

# Full-problem constants
B, S, D, H = 2, 2048, 1024, 16
DK = D // H  # 64
NCORES = 8
HPC = 4          # heads per core
DKC = HPC * DK   # 256 dk dims per core
P = 128
QC = 512         # tokens per chunk
NCH = S // QC    # 4 chunks
KPC = QC // P    # 4 k-tiles per chunk
NF = D // P      # 8 feature tiles
WSCALE = 32.0    # fp8 weight pre-scale

SPLIT_PROJ = True  # fp8 DoubleRow split projections (False -> fp16 proj)
DEBUG = False      # add debug dram dumps of intermediates

_PROGRAM_CACHE = {}
LAST = {}

F8NP = ml_dtypes.float8_e4m3


# ---------------------------------------------------------------------------
# Device program
# ---------------------------------------------------------------------------

def _mha_body(ctx, tc, io):
    from concourse import mybir

    F8 = mybir.dt.float8e4
    F16 = mybir.dt.float16
    F32 = mybir.dt.float32
    Exp = mybir.ActivationFunctionType.Exp
    DR = mybir.MatmulPerfMode.DoubleRow
    MUL = mybir.AluOpType.mult
    ADD = mybir.AluOpType.add

    nc = tc.nc

    consts = ctx.enter_context(tc.tile_pool(name="consts", bufs=1))
    xs = ctx.enter_context(tc.tile_pool(name="xs", bufs=1))
    persist = ctx.enter_context(tc.tile_pool(name="persist", bufs=1))
    pts = ctx.enter_context(tc.tile_pool(name="pts", bufs=4))
    wouts = ctx.enter_context(tc.tile_pool(name="wouts", bufs=4))
    pspool = ctx.enter_context(tc.tile_pool(name="ps", bufs=1, space="PSUM"))

    # ---- constants & inputs (DMA order = first-use order) ----------------
    # chunk-0 prologue data first (wq/wk/wv + first-chunk x), tiny biases,
    # then the rest of x, then late-use constants (tri/eye/wo).
    if SPLIT_PROJ:
        wdt, wshape = F8, [P, NF, 2, DKC]
    else:
        wdt, wshape = F16, [P, NF, 1, DKC]
    xdt = F8 if SPLIT_PROJ else F16
    nsel = 2 if SPLIT_PROJ else 1

    bx0, bxr = {}, {}
    bq_sb = consts.tile([P, 2], F32, tag="bq", name="bq_sb")
    nc.sync.dma_start(bq_sb[:], io["bq"])
    bk_sb = consts.tile([P, 2], F32, tag="bk", name="bk_sb")
    nc.sync.dma_start(bk_sb[:], io["bk"])
    w_sbs = {}
    for wnm, xnm in (("wq", "q"), ("wk", "k"), ("wv", "v")):
        w_sbs[wnm] = consts.tile(wshape, wdt, tag=wnm, name=wnm + "_sb")
        nc.sync.dma_start(w_sbs[wnm][:], io[wnm])
        t0 = xs.tile([P, NF, nsel, QC], xdt, tag=f"x{xnm}0", name=f"x{xnm}0")
        nc.sync.dma_start(t0[:], io["x" + xnm][:, :, :, 0:QC])
        bx0[xnm] = t0
    wq_sb, wk_sb, wv_sb = w_sbs["wq"], w_sbs["wk"], w_sbs["wv"]
    tri_sb = consts.tile([P, P], F16, tag="tri", name="tri_sb")
    nc.sync.dma_start(tri_sb[:], io["tri"])
    eye_sb = consts.tile([P, P], F16, tag="eye", name="eye_sb")
    nc.sync.dma_start(eye_sb[:], io["eye"])
    for nm in ("q", "k", "v"):
        bxr[nm] = xs.tile([P, NF, nsel, S - QC], xdt, tag=f"x{nm}r",
                          name=f"x{nm}r")
    # per-chunk input DMAs, interleaved q/k/v, in consumption order
    for j in range(1, NCH):
        for nm in ("q", "k", "v"):
            nc.sync.dma_start(
                bxr[nm][:, :, :, (j - 1) * QC:j * QC],
                io["x" + nm][:, :, :, j * QC:(j + 1) * QC])
    wo_sb = consts.tile([P, 2, NF, P], F16, tag="wo", name="wo_sb")
    nc.sync.dma_start(wo_sb[:], io["wo"])

    def xsl(nm, lo, hi):
        """[P, NF, nsel, hi-lo] view of input nm tokens [lo:hi)."""
        if hi <= QC:
            return bx0[nm][:, :, :, lo:hi]
        return bxr[nm][:, :, :, lo - QC:hi - QC]

    def ps_tile(tag, shape=None, dtype=F32, bufs=None):
        shape = shape or [P, QC]
        return pspool.tile(shape, dtype, tag=tag, name=tag, bufs=bufs)

    # ---- projection units (queued as PE fillers) -------------------------
    def mk_proj_qk_unit(j, w_sb, b_sb, out, t, srcnm):
        """One psum tile of a Q/K projection: matmuls + biased fp16 evict."""
        co = j * QC

        def go():
            pp = ps_tile("aux", bufs=2)
            if SPLIT_PROJ:
                first = True
                for hh in range(2):  # token halves (moving free <= 512)
                    ts0, ts1 = co + hh * 256, co + (hh + 1) * 256
                    for f in range(0, NF, 2):  # main terms: hi x hi pairs
                        nc.tensor.matmul(
                            pp[:, hh * 256:(hh + 1) * 256],
                            w_sb[:, f:f + 2, 0, t * P:(t + 1) * P],
                            xsl(srcnm, ts0, ts1)[:, f:f + 2, 1, :],
                            start=first, stop=False, perf_mode=DR,
                            skip_group_check=True)
                        first = False
                    for f in range(NF):  # cross terms: (hi,lo)x(lo,hi)
                        nc.tensor.matmul(
                            pp[:, hh * 256:(hh + 1) * 256],
                            w_sb[:, f, 0:2, t * P:(t + 1) * P],
                            xsl(srcnm, ts0, ts1)[:, f, 0:2, :],
                            start=False, stop=(f == NF - 1 and hh == 1),
                            perf_mode=DR, skip_group_check=True)
                scl = 1.0 / WSCALE
            else:
                for f in range(NF):
                    nc.tensor.matmul(
                        pp[:], w_sb[:, f, 0, t * P:(t + 1) * P],
                        xsl(srcnm, co, co + QC)[:, f, 0, :],
                        start=(f == 0), stop=(f == NF - 1))
                scl = 1.0
            nc.vector.tensor_scalar(out[:, t, :], pp[:], scl,
                                    b_sb[:, t:t + 1], MUL, ADD)
        return go

    def mk_proj_v_unit(j, t4):
        """One 128-token tile of the V projection -> vt tile (token-major)."""
        co = j * QC

        def go():
            pp = ps_tile("aux", [P, DKC], bufs=2)
            ts0, ts1 = co + t4 * P, co + (t4 + 1) * P
            if SPLIT_PROJ:
                first = True
                for f in range(0, NF, 2):
                    nc.tensor.matmul(
                        pp[:], xsl("v", ts0, ts1)[:, f:f + 2, 1, :],
                        wv_sb[:, f:f + 2, 0, :],
                        start=first, stop=False, perf_mode=DR,
                        skip_group_check=True)
                    first = False
                for f in range(NF):
                    nc.tensor.matmul(
                        pp[:], xsl("v", ts0, ts1)[:, f, 0:2, :],
                        wv_sb[:, f, 0:2, :],
                        start=False, stop=(f == NF - 1),
                        perf_mode=DR, skip_group_check=True)
                scl = 1.0 / WSCALE
            else:
                for f in range(NF):
                    nc.tensor.matmul(
                        pp[:], xsl("v", ts0, ts1)[:, f, 0, :],
                        wv_sb[:, f, 0, :],
                        start=(f == 0), stop=(f == NF - 1))
                scl = 1.0
            kt = j * KPC + t4
            vt = vt_tiles[kt]
            # ones column: PV's column 64 accumulates the softmax sums
            nc.vector.memset(vt[:, :, 64:65], 1.0)
            nc.vector.tensor_scalar_mul(
                vt[:, :, 0:64], pp[:].rearrange("p (h d) -> p h d", d=64), scl)
        return go

    qt = {}
    kt_tiles = {}
    vt_tiles = {}
    # cross-segment software pipeline state: the previous event's PV
    # emission and the previous segment's normalize hook
    pending = {"pv": None, "post": None}

    # Filler queues of PE closures drained inside ACT-bound attention
    # k-loops: `proj_fillers` (next chunk's projections + ctx transposes —
    # must drain promptly) and `wo_fillers` (Wo matmuls + output DMA of
    # finished chunks — deferred into the LAST chunk's long k-loop, which
    # is the only ACT-bound phase with spare PE time).
    proj_fillers = []
    wo_fillers = []

    def drain_filler(n=1, wo_ok=False):
        for _ in range(n):
            if proj_fillers:
                proj_fillers.pop(0)()
            elif wo_ok and wo_fillers:
                wo_fillers.pop(0)()

    def queue_proj_qk(jn):
        for (w_sb, b_sb, tag, srcnm) in ((wq_sb, bq_sb, f"qt{jn % 2}", "q"),
                                         (wk_sb, bk_sb, f"kt{jn}", "k")):
            out = persist.tile([P, 2, QC], F16, tag=tag, name=tag)
            if srcnm == "q":
                qt[jn % 2] = out
            else:
                kt_tiles[jn] = out
            for t in range(2):
                proj_fillers.append(
                    mk_proj_qk_unit(jn, w_sb, b_sb, out, t, srcnm))

    def queue_proj_v(jn):
        for t4 in range(KPC):
            kt = jn * KPC + t4
            vt_tiles[kt] = persist.tile([P, HPC, 65], F16, tag=f"vt{kt}",
                                        name=f"vt{kt}")
            proj_fillers.append(mk_proj_v_unit(jn, t4))

    def queue_proj(jn):
        queue_proj_qk(jn)
        queue_proj_v(jn)

    # ---- attention pass (2 heads) ---------------------------------------
    def emit_pass(j, p):
        """Attention for chunk j, heads {2p, 2p+1}. Returns pvacc tiles."""
        nkt = KPC * (j + 1)
        qtile = qt[j % 2]
        pv = [ps_tile(f"pv{l}", [P, KPC, 80]) for l in range(2)]

        def head_slices(l):
            """(dim-tile t, partition offset) for local head l of pass p."""
            hh = 2 * p + l
            return hh // 2, (hh % 2) * 64

        def emit_qk_exp(kt):
            jk = kt // KPC          # source chunk of this k-tile
            ko = (kt % KPC) * P
            tdiag = kt - KPC * j    # >=0 on the diagonal chunk
            ktile = kt_tiles[jk]
            sw = ps_tile("swA" if kt % 2 == 0 else "swB", [P, 2 * QC])
            c0 = max(tdiag, 0) * P
            for l in range(2):
                t, po = head_slices(l)
                nc.tensor.matmul(
                    sw[:, l * QC + c0:(l + 1) * QC],
                    ktile[po:po + 64, t, ko:ko + P],
                    qtile[po:po + 64, t, c0:QC],
                    start=True, stop=True)
            pt = pts.tile([P, 2 * QC], F16, tag="pt", name="pt")
            if c0:
                # one strided exp covering both heads' live columns
                swv = sw[:].rearrange("p (l q) -> p l q", l=2)[:, :, c0:QC]
                ptv = pt[:].rearrange("p (l q) -> p l q", l=2)[:, :, c0:QC]
                nc.scalar.activation(ptv, swv, Exp, scale=0.125)
            else:
                nc.scalar.activation(pt[:], sw[:], Exp, scale=0.125)
            if tdiag >= 0:
                # masked-diagonal multiply on the (otherwise idle) Pool
                # engine so exp-dependent work never blocks DVE's evictions
                for l in range(2):
                    nc.gpsimd.tensor_mul(pt[:, l * QC + c0:l * QC + c0 + P],
                                         pt[:, l * QC + c0:l * QC + c0 + P],
                                         tri_sb[:])
            if DEBUG and j == 0 and p == 0 and kt == 0:
                nc.sync.dma_start(io["dbg_pt"], pt[:])

            def emit_pv():
                vt = vt_tiles[kt]
                for l in range(2):
                    hh = 2 * p + l
                    for t in range(max(tdiag, 0), KPC):
                        nc.tensor.matmul(
                            pv[l][:, t, 0:65],
                            pt[:, l * QC + t * P:l * QC + (t + 1) * P],
                            vt[:, hh, :],
                            start=(kt == 0 and t == 0),
                            stop=(kt == KPC * j + t),
                            skip_group_check=True)
            return emit_pv

        # Globally software-pipelined: the PV of the previous event — which
        # may belong to the PREVIOUS segment — is emitted after this event's
        # QK+exp, so the ACT engine never waits at segment boundaries. The
        # previous segment's normalize hook runs right after its last PV.
        wo_ok = (j >= NCH - 2)
        for kt in range(nkt):
            pv_next = emit_qk_exp(kt)
            if pending["pv"] is not None:
                pending["pv"]()
            if pending["post"] is not None:
                pending["post"]()
                pending["post"] = None
            drain_filler(2 if len(proj_fillers) > 8 else 1, wo_ok)
            pending["pv"] = pv_next
        return pv

    # ---- normalize + transpose + Wo for one chunk ------------------------
    def emit_norm(j, p, pv):
        """Normalize pass p of chunk j into ctx_sb (DVE), return ctx_sb."""
        cs = persist.tile([P, 2, KPC, 64], F16, tag=f"ctxs{p}",
                          name=f"ctxs{p}")
        for l in range(2):
            rc = persist.tile([P, KPC], F32, tag=f"rc{p}{l}", name=f"rc{p}{l}")
            nc.vector.reciprocal(rc[:], pv[l][:, :, 64:65].squeeze(2))
            for t in range(KPC):
                nc.vector.tensor_scalar_mul(cs[:, l, t, :], pv[l][:, t, 0:64],
                                            rc[:, t:t + 1])
        return cs

    def mk_transp(cs, ctx_t, p):
        def go():
            trp = ps_tile("aux", [P, QC], F16, bufs=2)
            for l in range(2):
                for t in range(KPC):
                    nc.tensor.matmul(
                        trp[l * 64:(l + 1) * 64, t * P:(t + 1) * P],
                        cs[:, l, t, :], eye_sb[:],
                        is_transpose=True,
                        start=(t == 0), stop=(t == KPC - 1),
                        skip_group_check=True,
                        tile_position=(0, l * 64))
            nc.vector.tensor_copy(ctx_t[:, p, :], trp[:])
        return go

    def mk_wo(ctx_t, wout, m, act_ok):
        def go():
            po = ps_tile("aux", [P, QC], bufs=2)
            for t in range(2):
                nc.tensor.matmul(po[:], wo_sb[:, t, m, :], ctx_t[:, t, :],
                                 start=(t == 0), stop=(t == 1))
            if act_ok and m % 2:
                # tail chunk: ACT is idle there, share the eviction load
                nc.scalar.copy(wout[:, m, :], po[:])
            else:
                nc.vector.tensor_copy(wout[:, m, :], po[:])
        return go

    def mk_out_dma(j, wout):
        def go():
            nc.gpsimd.dma_start(
                io["out_t"].rearrange("(m p) s -> p m s", p=P)
                [:, :, j * QC:(j + 1) * QC], wout[:])
        return go

    # ---- main loop -------------------------------------------------------
    # chunk 0's projections run up front; chunk j+1's are drained as PE
    # fillers inside chunk j's ACT-bound attention, together with chunk
    # j-1/j's transpose + Wo + output-DMA epilogue.
    queue_proj_qk(0)
    while proj_fillers:
        proj_fillers.pop(0)()
    queue_proj_v(0)
    for j in range(NCH):
        if j + 1 < NCH:
            queue_proj(j + 1)
        ctx_t = persist.tile([P, 2, QC], F16, tag=f"ctxt{j}",
                             name=f"ctxt{j}")
        wout = wouts.tile([P, NF, QC], F16, tag="wout", name="wout")
        for p in range(2):
            pv = emit_pass(j, p)

            def mk_post(j=j, p=p, pv=pv, ctx_t=ctx_t, wout=wout):
                def post():
                    cs = emit_norm(j, p, pv)
                    proj_fillers.append(mk_transp(cs, ctx_t, p))
                    if p == 1:
                        for m in range(NF):
                            wo_fillers.append(
                                mk_wo(ctx_t, wout, m, j == NCH - 1))
                        wo_fillers.append(mk_out_dma(j, wout))
                return post
            pending["post"] = mk_post()
        # chunk j+1's attention reads qt/kt/vt(j+1): those projections must
        # be fully emitted before the next emit_pass
        while proj_fillers:
            proj_fillers.pop(0)()
    if pending["pv"] is not None:
        pending["pv"]()
        pending["pv"] = None
    if pending["post"] is not None:
        pending["post"]()
        pending["post"] = None
    while proj_fillers:
        proj_fillers.pop(0)()
    while wo_fillers:
        wo_fillers.pop(0)()


def build_program():
    import concourse.tile as tile
    from concourse import bacc, mybir
    from contextlib import ExitStack

    F8 = mybir.dt.float8e4
    F16 = mybir.dt.float16
    F32 = mybir.dt.float32

    nc = bacc.Bacc("TRN2", target_bir_lowering=False, debug=False)
    xdt = F8 if SPLIT_PROJ else F16
    nsel = 2 if SPLIT_PROJ else 1
    wdt = F8 if SPLIT_PROJ else F16
    io = {
        "xq": nc.dram_tensor("xq", [P, NF, nsel, S], xdt,
                             kind="ExternalInput").ap(),
        "xk": nc.dram_tensor("xk", [P, NF, nsel, S], xdt,
                             kind="ExternalInput").ap(),
        "xv": nc.dram_tensor("xv", [P, NF, nsel, S], xdt,
                             kind="ExternalInput").ap(),
        "wq": nc.dram_tensor("wq", [P, NF, nsel, DKC], wdt,
                             kind="ExternalInput").ap(),
        "wk": nc.dram_tensor("wk", [P, NF, nsel, DKC], wdt,
                             kind="ExternalInput").ap(),
        "wv": nc.dram_tensor("wv", [P, NF, nsel, DKC], wdt,
                             kind="ExternalInput").ap(),
        "wo": nc.dram_tensor("wo", [P, 2, NF, P], F16,
                             kind="ExternalInput").ap(),
        "tri": nc.dram_tensor("tri", [P, P], F16, kind="ExternalInput").ap(),
        "eye": nc.dram_tensor("eye", [P, P], F16, kind="ExternalInput").ap(),
        "bq": nc.dram_tensor("bq", [P, 2], F32, kind="ExternalInput").ap(),
        "bk": nc.dram_tensor("bk", [P, 2], F32, kind="ExternalInput").ap(),
        "out_t": nc.dram_tensor("out_t", [D, S], F16,
                                kind="ExternalOutput").ap(),
    }
    if DEBUG:
        io["dbg_qt"] = nc.dram_tensor("dbg_qt", [P, 2, QC], F16,
                                      kind="ExternalOutput").ap()
        io["dbg_kt"] = nc.dram_tensor("dbg_kt", [P, 2, QC], F16,
                                      kind="ExternalOutput").ap()
        io["dbg_vt"] = nc.dram_tensor("dbg_vt", [P, HPC, 65], F16,
                                      kind="ExternalOutput").ap()
        io["dbg_pt"] = nc.dram_tensor("dbg_pt", [P, 2 * QC], F16,
                                      kind="ExternalOutput").ap()
        io["dbg_ctxs"] = nc.dram_tensor("dbg_ctxs", [P, 2, KPC, 64], F16,
                                        kind="ExternalOutput").ap()
    with tile.TileContext(nc) as tc, ExitStack() as ctx:
        _mha_body(ctx, tc, io)
    nc.compile()
    return nc


# ---------------------------------------------------------------------------
# Host side
# ---------------------------------------------------------------------------

def _np_reference(query, key, value, mask, Wq, bq, Wk, bk, Wv, bv, Wo, bo):
    q = (query.reshape(-1, D) @ Wq + bq).reshape(B, S, H, DK).transpose(0, 2, 1, 3)
    k = (key.reshape(-1, D) @ Wk + bk).reshape(B, S, H, DK).transpose(0, 2, 1, 3)
    v = (value.reshape(-1, D) @ Wv + bv).reshape(B, S, H, DK).transpose(0, 2, 1, 3)
    scores = np.einsum("bhqd,bhkd->bhqk", q, k) / math.sqrt(DK)
    scores = np.where(mask[:, None, :, :] == 0, np.float32(-1e9), scores)
    scores -= scores.max(axis=-1, keepdims=True)
    p = np.exp(scores)
    p /= p.sum(axis=-1, keepdims=True)
    x = np.einsum("bhqk,bhkd->bhqd", p, v)
    x = x.transpose(0, 2, 1, 3).reshape(B, -1, D)
    return (x @ Wo + bo).astype(np.float32)


def _split8(x):
    """fp32 -> (lo, hi) fp8e4 with hi = fp8(x), lo = fp8(x - hi)."""
    hi = x.astype(F8NP)
    lo = (x - hi.astype(np.float32)).astype(F8NP)
    return lo, hi


def _xlayout(x):
    """[S, D] batch slice -> [128, NF, nsel, S]: [p, f, sel, t] with
    sel = (lo, hi) fp8 split (or a single fp16 plane)."""
    xt = np.ascontiguousarray(x.T)  # [D, S]
    xt = xt.reshape(NF, P, S).transpose(1, 0, 2)  # [p, f, t]
    if not SPLIT_PROJ:
        return np.ascontiguousarray(xt[:, :, None, :]).astype(np.float16)
    lo, hi = _split8(xt.astype(np.float32))
    out = np.empty((P, NF, 2, S), F8NP)
    out[:, :, 0, :] = lo
    out[:, :, 1, :] = hi
    return out


def _wlayout(w):
    """[D, DKC] weight slice -> [128, NF, nsel, DKC]: [p, f, sel, m] with
    sel = (hi, lo) of the x32-scaled weight (or single fp16 plane)."""
    wr = w.reshape(NF, P, DKC).transpose(1, 0, 2)  # [p, f, m]
    if not SPLIT_PROJ:
        return np.ascontiguousarray(wr[:, :, None, :]).astype(np.float16)
    lo, hi = _split8(wr.astype(np.float32) * WSCALE)
    out = np.empty((P, NF, 2, DKC), F8NP)
    out[:, :, 0, :] = hi
    out[:, :, 1, :] = lo
    return out


def _shard_inputs(query, key, value, Wq, bq, Wk, bk, Wv, Wo):
    idx = np.arange(P)
    tri = (idx[:, None] <= idx[None, :]).astype(np.float16)  # [k, q] k<=q
    eye = np.eye(P, dtype=np.float16)
    in_maps = []
    xcache = {}
    for c in range(NCORES):
        bb, g = c // 4, c % 4
        sl = slice(g * DKC, (g + 1) * DKC)
        if bb not in xcache:
            xcache[bb] = {
                "xq": _xlayout(query[bb]),
                "xk": _xlayout(key[bb]),
                "xv": _xlayout(value[bb]),
            }
        wo = Wo[sl, :].reshape(2, P, NF, P).transpose(1, 0, 2, 3)
        in_maps.append({
            **xcache[bb],
            "wq": _wlayout(Wq[:, sl]),
            "wk": _wlayout(Wk[:, sl]),
            "wv": _wlayout(Wv[:, sl]),
            "wo": np.ascontiguousarray(wo).astype(np.float16),
            "tri": tri,
            "eye": eye,
            "bq": np.ascontiguousarray(bq[sl].reshape(2, P).T).astype(np.float32),
            "bk": np.ascontiguousarray(bk[sl].reshape(2, P).T).astype(np.float32),
        })
    return in_maps


def kernel(**inputs):
    query = np.asarray(inputs["query"], np.float32)
    key = np.asarray(inputs["key"], np.float32)
    value = np.asarray(inputs["value"], np.float32)
    mask = np.asarray(inputs["mask"])
    Wq = np.asarray(inputs["Wq"], np.float32)
    bq = np.asarray(inputs["bq"], np.float32)
    Wk = np.asarray(inputs["Wk"], np.float32)
    bk = np.asarray(inputs["bk"], np.float32)
    Wv = np.asarray(inputs["Wv"], np.float32)
    bv = np.asarray(inputs["bv"], np.float32)
    Wo = np.asarray(inputs["Wo"], np.float32)
    bo = np.asarray(inputs["bo"], np.float32)

    tril = np.tril(np.ones((S, S), np.int8))
    if mask.shape != (B, S, S) or not np.array_equal(
            (mask != 0).astype(np.int8), np.broadcast_to(tril, (B, S, S))):
        return _np_reference(query, key, value, mask,
                             Wq, bq, Wk, bk, Wv, bv, Wo, bo)

    in_maps = _shard_inputs(query, key, value, Wq, bq, Wk, bk, Wv, Wo)
    outs = _run_spmd(in_maps)  # [8, D, S]

    outs = outs.astype(np.float32)
    corr = (bv @ Wo + bo)[None, :]
    res = np.empty((B, S, D), np.float32)
    for bb in range(B):
        acc = outs[bb * 4:(bb + 1) * 4].sum(axis=0)  # [D, S]
        res[bb] = acc.T + corr
    return res


def _get_exec():
    if "exec" in _PROGRAM_CACHE:
        return _PROGRAM_CACHE["exec"]
    import jax
    from jax.sharding import Mesh, PartitionSpec
    from jax.experimental.shard_map import shard_map
    import concourse.mybir as mybir
    from concourse import bass2jax

    nc = build_program()
    _PROGRAM_CACHE["nc"] = nc
    bass2jax.install_neuronx_cc_hook()
    partition_name = nc.partition_id_tensor.name if nc.partition_id_tensor else None
    in_names, out_names, out_avals, zero_outs = [], [], [], []
    for alloc in nc.m.functions[0].allocations:
        if not isinstance(alloc, mybir.MemoryLocationSet):
            continue
        name = alloc.memorylocations[0].name
        if alloc.kind == "ExternalInput":
            if name != partition_name:
                in_names.append(name)
        elif alloc.kind == "ExternalOutput":
            out_names.append(name)
            shape = tuple(alloc.tensor_shape)
            dtype = mybir.dt.np(alloc.dtype)
            out_avals.append(jax.core.ShapedArray(shape, dtype))
            zero_outs.append(np.zeros(shape, dtype))
    n_params = len(in_names)
    all_in_names = list(in_names) + list(out_names)
    if partition_name is not None:
        all_in_names.append(partition_name)

    def _body(*args):
        operands = list(args)
        if partition_name is not None:
            operands.append(bass2jax.partition_id_tensor())
        return tuple(bass2jax._bass_exec_p.bind(
            *operands,
            out_avals=tuple(out_avals),
            in_names=tuple(all_in_names),
            out_names=tuple(out_names),
            lowering_input_output_aliases=(),
            sim_require_finite=True,
            sim_require_nnan=True,
            nc=nc,
        ))

    devices = jax.devices()[:NCORES]
    assert len(devices) >= NCORES, f"need {NCORES} cores, have {len(devices)}"
    mesh = Mesh(np.asarray(devices[:NCORES]), ("core",))
    fn = jax.jit(
        shard_map(_body, mesh=mesh,
                  in_specs=(PartitionSpec("core"),) * (n_params + len(zero_outs)),
                  out_specs=(PartitionSpec("core"),) * len(out_names),
                  check_rep=False),
        donate_argnums=tuple(range(n_params, n_params + len(out_names))),
        keep_unused=True)
    _PROGRAM_CACHE["exec"] = (fn, in_names, zero_outs)
    return _PROGRAM_CACHE["exec"]


def _run_spmd(in_maps):
    fn, in_names, zero_outs = _get_exec()
    concat_in = [np.concatenate([np.asarray(in_maps[c][nm])
                                 for c in range(NCORES)], axis=0)
                 for nm in in_names]
    concat_zero = [np.zeros((NCORES * z.shape[0], *z.shape[1:]), z.dtype)
                   for z in zero_outs]
    out = fn(*concat_in, *concat_zero)
    LAST["out"] = out
    return np.asarray(out[0]).reshape(NCORES, D, S)


# revision 4
# speedup vs baseline: 1.4743x; 1.0197x over previous
"""Trainium2 Bass kernel for 16-head causal multi-head attention (v2).

Problem: B=2, S=2048, D=1024, H=16 (head dim 64), causal mask.
    out = softmax((XqWq+bq)(XkWk+bk)^T / 8, causal) (XvWv+bv) Wo + bo

Sharding: 8 cores = 2 batches x 4 head-groups. Core c owns batch c//4 and
heads 4*(c%4)..4*(c%4)+3 (dk dims 256): Wq/Wk/Wv column-sliced, Wo
row-sliced. Each core computes a partial y^T [1024, 2048] for its batch;
host sums the 4 partials per batch and adds (bv @ Wo + bo).

Device-side design (per core):
  - Projections in fp8e4 DoubleRow with residual hi/lo split:
    q = X_hi@W_hi + (X_lo@W_hi + X_hi@W_lo), W pre-scaled by 32 so its fp8
    values are unit-variance; the 1/32 is folded into the PSUM eviction.
    Cost model charge: 0.75x of fp16, at ~2e-3 end-to-end error.
  - QK in fp16, scores transposed S^T[k, q] so exp output P^T is ready as
    the PV stationary operand.
  - PV "swapped": out ctx[q, dk+1] with P^T [k, q-subtile] stationary and
    V-augmented [k, 65] moving -> charge 65 rows per (ktile, qsub, head)
    instead of 512 (2x PE saving); the ones column (x32) yields softmax
    sums so normalization is a per-partition (per-token) scalar multiply.
  - ctx is transposed back (PE is_transpose matmuls into one psum bank)
    for the fp16 Wo matmuls; Wo/transpose work of chunk j is emitted as
    PE filler inside chunk j+1's ACT-bound attention loop.
  - 4 heads are processed as 2 passes of 2 heads to fit PSUM:
    banks = scores dbuf (2x2) + PV accum (2) + aux dbuf (2).
"""

import math

import numpy as np
import ml_dtypes

# Full-problem constants
B, S, D, H = 2, 2048, 1024, 16
DK = D // H  # 64
NCORES = 8
HPC = 4          # heads per core
DKC = HPC * DK   # 256 dk dims per core
P = 128
QC = 512         # tokens per chunk
NCH = S // QC    # 4 chunks
KPC = QC // P    # 4 k-tiles per chunk
NF = D // P      # 8 feature tiles
WSCALE = 32.0    # fp8 weight pre-scale

SPLIT_PROJ = True  # fp8 DoubleRow split projections (False -> fp16 proj)
DEBUG = False      # add debug dram dumps of intermediates

_PROGRAM_CACHE = {}
LAST = {}

F8NP = ml_dtypes.float8_e4m3


# ---------------------------------------------------------------------------
# Device program
# ---------------------------------------------------------------------------

def _mha_body(ctx, tc, io):
    from concourse import mybir

    F8 = mybir.dt.float8e4
    F16 = mybir.dt.float16
    F32 = mybir.dt.float32
    Exp = mybir.ActivationFunctionType.Exp
    DR = mybir.MatmulPerfMode.DoubleRow
    MUL = mybir.AluOpType.mult
    ADD = mybir.AluOpType.add

    nc = tc.nc

    consts = ctx.enter_context(tc.tile_pool(name="consts", bufs=1))
    xs = ctx.enter_context(tc.tile_pool(name="xs", bufs=1))
    persist = ctx.enter_context(tc.tile_pool(name="persist", bufs=1))
    pts = ctx.enter_context(tc.tile_pool(name="pts", bufs=4))
    wouts = ctx.enter_context(tc.tile_pool(name="wouts", bufs=4))
    pspool = ctx.enter_context(tc.tile_pool(name="ps", bufs=1, space="PSUM"))

    # ---- constants & inputs (DMA order = first-use order) ----------------
    # chunk-0 prologue data first (wq/wk/wv + first-chunk x), tiny biases,
    # then the rest of x, then late-use constants (tri/eye/wo).
    if SPLIT_PROJ:
        wdt, wshape = F8, [P, NF, 2, DKC]
    else:
        wdt, wshape = F16, [P, NF, 1, DKC]
    xdt = F8 if SPLIT_PROJ else F16
    nsel = 2 if SPLIT_PROJ else 1

    bx0, bxr = {}, {}
    bq_sb = consts.tile([P, 2], F32, tag="bq", name="bq_sb")
    nc.sync.dma_start(bq_sb[:], io["bq"])
    bk_sb = consts.tile([P, 2], F32, tag="bk", name="bk_sb")
    nc.sync.dma_start(bk_sb[:], io["bk"])
    w_sbs = {}
    for wnm, xnm in (("wq", "q"), ("wk", "k"), ("wv", "v")):
        w_sbs[wnm] = consts.tile(wshape, wdt, tag=wnm, name=wnm + "_sb")
        nc.sync.dma_start(w_sbs[wnm][:], io[wnm])
        t0 = xs.tile([P, NF, nsel, QC], xdt, tag=f"x{xnm}0", name=f"x{xnm}0")
        if xnm == "k":
            nc.sync.dma_start(t0[:, 0:NF // 2], io["xk"][:, 0:NF // 2, :, 0:QC])
            nc.sync.dma_start(t0[:, NF // 2:], io["xk"][:, NF // 2:, :, 0:QC])
        else:
            nc.sync.dma_start(t0[:], io["x" + xnm][:, :, :, 0:QC])
        bx0[xnm] = t0
    wq_sb, wk_sb, wv_sb = w_sbs["wq"], w_sbs["wk"], w_sbs["wv"]
    tri_sb = consts.tile([P, P], F16, tag="tri", name="tri_sb")
    nc.sync.dma_start(tri_sb[:], io["tri"])
    eye_sb = consts.tile([P, P], F16, tag="eye", name="eye_sb")
    nc.sync.dma_start(eye_sb[:], io["eye"])
    for nm in ("q", "k", "v"):
        bxr[nm] = xs.tile([P, NF, nsel, S - QC], xdt, tag=f"x{nm}r",
                          name=f"x{nm}r")
    # per-chunk input DMAs, interleaved q/k/v, in consumption order
    for j in range(1, NCH):
        for nm in ("q", "k", "v"):
            nc.sync.dma_start(
                bxr[nm][:, :, :, (j - 1) * QC:j * QC],
                io["x" + nm][:, :, :, j * QC:(j + 1) * QC])
    wo_sb = consts.tile([P, 2, NF, P], F16, tag="wo", name="wo_sb")
    nc.sync.dma_start(wo_sb[:], io["wo"])

    def xsl(nm, lo, hi):
        """[P, NF, nsel, hi-lo] view of input nm tokens [lo:hi)."""
        if hi <= QC:
            return bx0[nm][:, :, :, lo:hi]
        return bxr[nm][:, :, :, lo - QC:hi - QC]

    def ps_tile(tag, shape=None, dtype=F32, bufs=None):
        shape = shape or [P, QC]
        return pspool.tile(shape, dtype, tag=tag, name=tag, bufs=bufs)

    # ---- projection units (queued as PE fillers) -------------------------
    def mk_proj_qk_unit(j, w_sb, b_sb, out, t, srcnm):
        """One psum tile of a Q/K projection: matmuls + biased fp16 evict."""
        co = j * QC

        def go():
            pp = ps_tile("aux", bufs=2)
            if SPLIT_PROJ:
                first = True
                for hh in range(2):  # token halves (moving free <= 512)
                    ts0, ts1 = co + hh * 256, co + (hh + 1) * 256
                    for f in range(0, NF, 2):  # main terms: hi x hi pairs
                        nc.tensor.matmul(
                            pp[:, hh * 256:(hh + 1) * 256],
                            w_sb[:, f:f + 2, 0, t * P:(t + 1) * P],
                            xsl(srcnm, ts0, ts1)[:, f:f + 2, 1, :],
                            start=first, stop=False, perf_mode=DR,
                            skip_group_check=True)
                        first = False
                    for f in range(NF):  # cross terms: (hi,lo)x(lo,hi)
                        nc.tensor.matmul(
                            pp[:, hh * 256:(hh + 1) * 256],
                            w_sb[:, f, 0:2, t * P:(t + 1) * P],
                            xsl(srcnm, ts0, ts1)[:, f, 0:2, :],
                            start=False, stop=(f == NF - 1 and hh == 1),
                            perf_mode=DR, skip_group_check=True)
                scl = 1.0 / WSCALE
            else:
                for f in range(NF):
                    nc.tensor.matmul(
                        pp[:], w_sb[:, f, 0, t * P:(t + 1) * P],
                        xsl(srcnm, co, co + QC)[:, f, 0, :],
                        start=(f == 0), stop=(f == NF - 1))
                scl = 1.0
            nc.vector.tensor_scalar(out[:, t, :], pp[:], scl,
                                    b_sb[:, t:t + 1], MUL, ADD)
        return go

    def mk_proj_v_unit(j, t4):
        """One 128-token tile of the V projection -> vt tile (token-major)."""
        co = j * QC

        def go():
            pp = ps_tile("aux", [P, DKC], bufs=2)
            ts0, ts1 = co + t4 * P, co + (t4 + 1) * P
            if SPLIT_PROJ:
                first = True
                for f in range(0, NF, 2):
                    nc.tensor.matmul(
                        pp[:], xsl("v", ts0, ts1)[:, f:f + 2, 1, :],
                        wv_sb[:, f:f + 2, 0, :],
                        start=first, stop=False, perf_mode=DR,
                        skip_group_check=True)
                    first = False
                for f in range(NF):
                    nc.tensor.matmul(
                        pp[:], xsl("v", ts0, ts1)[:, f, 0:2, :],
                        wv_sb[:, f, 0:2, :],
                        start=False, stop=(f == NF - 1),
                        perf_mode=DR, skip_group_check=True)
                scl = 1.0 / WSCALE
            else:
                for f in range(NF):
                    nc.tensor.matmul(
                        pp[:], xsl("v", ts0, ts1)[:, f, 0, :],
                        wv_sb[:, f, 0, :],
                        start=(f == 0), stop=(f == NF - 1))
                scl = 1.0
            kt = j * KPC + t4
            vt = vt_tiles[kt]
            # ones column: PV's column 64 accumulates the softmax sums
            nc.vector.memset(vt[:, :, 64:65], 1.0)
            nc.vector.tensor_scalar_mul(
                vt[:, :, 0:64], pp[:].rearrange("p (h d) -> p h d", d=64), scl)
        return go

    qt = {}
    kt_tiles = {}
    vt_tiles = {}
    # cross-segment software pipeline state: the previous event's PV
    # emission and the previous segment's normalize hook
    pending = {"pv": None, "post": None, "tick": 0}

    # Filler queues of PE closures drained inside ACT-bound attention
    # k-loops: `proj_fillers` (next chunk's projections + ctx transposes —
    # must drain promptly) and `wo_fillers` (Wo matmuls + output DMA of
    # finished chunks — deferred into the LAST chunk's long k-loop, which
    # is the only ACT-bound phase with spare PE time).
    proj_fillers = []
    wo_fillers = []

    def drain_filler(n=1, wo_ok=False):
        for _ in range(n):
            if proj_fillers:
                proj_fillers.pop(0)()
            elif wo_ok and wo_fillers:
                wo_fillers.pop(0)()

    def queue_proj_qk(jn):
        for (w_sb, b_sb, tag, srcnm) in ((wq_sb, bq_sb, f"qt{jn % 2}", "q"),
                                         (wk_sb, bk_sb, f"kt{jn}", "k")):
            out = persist.tile([P, 2, QC], F16, tag=tag, name=tag)
            if srcnm == "q":
                qt[jn % 2] = out
            else:
                kt_tiles[jn] = out
            for t in range(2):
                proj_fillers.append(
                    mk_proj_qk_unit(jn, w_sb, b_sb, out, t, srcnm))

    def queue_proj_v(jn):
        for t4 in range(KPC):
            kt = jn * KPC + t4
            vt_tiles[kt] = persist.tile([P, HPC, 65], F16, tag=f"vt{kt}",
                                        name=f"vt{kt}")
            proj_fillers.append(mk_proj_v_unit(jn, t4))

    def queue_proj(jn):
        queue_proj_qk(jn)
        queue_proj_v(jn)

    # ---- attention pass (2 heads) ---------------------------------------
    def emit_pass(j, p):
        """Attention for chunk j, heads {2p, 2p+1}. Returns pvacc tiles."""
        nkt = KPC * (j + 1)
        qtile = qt[j % 2]
        pv = [ps_tile(f"pv{l}", [P, KPC, 80]) for l in range(2)]

        def head_slices(l):
            """(dim-tile t, partition offset) for local head l of pass p."""
            hh = 2 * p + l
            return hh // 2, (hh % 2) * 64

        def emit_qk_exp(kt):
            jk = kt // KPC          # source chunk of this k-tile
            ko = (kt % KPC) * P
            tdiag = kt - KPC * j    # >=0 on the diagonal chunk
            ktile = kt_tiles[jk]
            sw = ps_tile("swA" if kt % 2 == 0 else "swB", [P, 2 * QC])
            c0 = max(tdiag, 0) * P
            for l in range(2):
                t, po = head_slices(l)
                nc.tensor.matmul(
                    sw[:, l * QC + c0:(l + 1) * QC],
                    ktile[po:po + 64, t, ko:ko + P],
                    qtile[po:po + 64, t, c0:QC],
                    start=True, stop=True)
            pt = pts.tile([P, 2 * QC], F16, tag="pt", name="pt")
            if c0:
                # one strided exp covering both heads' live columns
                swv = sw[:].rearrange("p (l q) -> p l q", l=2)[:, :, c0:QC]
                ptv = pt[:].rearrange("p (l q) -> p l q", l=2)[:, :, c0:QC]
                nc.scalar.activation(ptv, swv, Exp, scale=0.125)
            else:
                nc.scalar.activation(pt[:], sw[:], Exp, scale=0.125)
            if tdiag >= 0:
                # masked-diagonal multiply on the (otherwise idle) Pool
                # engine so exp-dependent work never blocks DVE's evictions
                for l in range(2):
                    nc.gpsimd.tensor_mul(pt[:, l * QC + c0:l * QC + c0 + P],
                                         pt[:, l * QC + c0:l * QC + c0 + P],
                                         tri_sb[:])
            if DEBUG and j == 0 and p == 0 and kt == 0:
                nc.sync.dma_start(io["dbg_pt"], pt[:])

            def emit_pv():
                vt = vt_tiles[kt]
                for l in range(2):
                    hh = 2 * p + l
                    for t in range(max(tdiag, 0), KPC):
                        nc.tensor.matmul(
                            pv[l][:, t, 0:65],
                            pt[:, l * QC + t * P:l * QC + (t + 1) * P],
                            vt[:, hh, :],
                            start=(kt == 0 and t == 0),
                            stop=(kt == KPC * j + t),
                            skip_group_check=True)
            return emit_pv

        # Globally software-pipelined: the PV of the previous event — which
        # may belong to the PREVIOUS segment — is emitted after this event's
        # QK+exp, so the ACT engine never waits at segment boundaries. The
        # previous segment's normalize hook runs right after its last PV.
        wo_ok = (j >= NCH - 2)
        for kt in range(nkt):
            pv_next = emit_qk_exp(kt)
            if pending["pv"] is not None:
                pending["pv"]()
            if pending["post"] is not None:
                pending["post"]()
                pending["post"] = None
            drain_filler(2 if len(proj_fillers) > 8 else 1, wo_ok)
            pending["pv"] = pv_next
        return pv

    # ---- normalize + transpose + Wo for one chunk ------------------------
    def emit_norm(j, p, pv):
        """Normalize pass p of chunk j into ctx_sb (DVE), return ctx_sb."""
        cs = persist.tile([P, 2, KPC, 64], F16, tag=f"ctxs{p}",
                          name=f"ctxs{p}")
        for l in range(2):
            rc = persist.tile([P, KPC], F32, tag=f"rc{p}{l}", name=f"rc{p}{l}")
            nc.vector.reciprocal(rc[:], pv[l][:, :, 64:65].squeeze(2))
            for t in range(KPC):
                nc.vector.tensor_scalar_mul(cs[:, l, t, :], pv[l][:, t, 0:64],
                                            rc[:, t:t + 1])
        return cs

    def mk_transp(cs, ctx_t, p):
        def go():
            trp = ps_tile("aux", [P, QC], F16, bufs=2)
            for l in range(2):
                for t in range(KPC):
                    nc.tensor.matmul(
                        trp[l * 64:(l + 1) * 64, t * P:(t + 1) * P],
                        cs[:, l, t, :], eye_sb[:],
                        is_transpose=True,
                        start=(t == 0), stop=(t == KPC - 1),
                        skip_group_check=True,
                        tile_position=(0, l * 64))
            nc.vector.tensor_copy(ctx_t[:, p, :], trp[:])
        return go

    def mk_wo(ctx_t, wout, m, act_ok):
        def go():
            if act_ok:
                # tail of the last chunk: score banks are free, use them to
                # deepen the psum rotation so Wo/evict fully pipeline
                tag = ("aux", "swA", "swB")[m % 3]
            else:
                tag = "aux"
            po = ps_tile(tag, [P, QC], bufs=2 if tag == "aux" else None)
            for t in range(2):
                nc.tensor.matmul(po[:], wo_sb[:, t, m, :], ctx_t[:, t, :],
                                 start=(t == 0), stop=(t == 1))
            if act_ok and m % 2:
                # tail chunk: ACT is idle there, share the eviction load
                nc.scalar.copy(wout[:, m, :], po[:])
            else:
                nc.vector.tensor_copy(wout[:, m, :], po[:])
        return go

    def mk_out_dma(j, wout, m0, m1):
        def go():
            nc.gpsimd.dma_start(
                io["out_t"].rearrange("(m p) s -> p m s", p=P)
                [:, m0:m1, j * QC:(j + 1) * QC], wout[:, m0:m1, :])
        return go

    # ---- main loop -------------------------------------------------------
    # chunk 0's projections run up front; chunk j+1's are drained as PE
    # fillers inside chunk j's ACT-bound attention, together with chunk
    # j-1/j's transpose + Wo + output-DMA epilogue.
    queue_proj_qk(0)
    while proj_fillers:
        proj_fillers.pop(0)()
    queue_proj_v(0)
    for j in range(NCH):
        if j + 1 < NCH:
            queue_proj(j + 1)
        ctx_t = persist.tile([P, 2, QC], F16, tag=f"ctxt{j}",
                             name=f"ctxt{j}")
        wout = wouts.tile([P, NF, QC], F16, tag="wout", name="wout")
        for p in range(2):
            pv = emit_pass(j, p)

            def mk_post(j=j, p=p, pv=pv, ctx_t=ctx_t, wout=wout):
                def post():
                    cs = emit_norm(j, p, pv)
                    proj_fillers.append(mk_transp(cs, ctx_t, p))
                    if p == 1:
                        for m in range(NF):
                            wo_fillers.append(
                                mk_wo(ctx_t, wout, m, j == NCH - 1))
                            if m == NF // 2 - 1:
                                wo_fillers.append(
                                    mk_out_dma(j, wout, 0, NF // 2))
                        wo_fillers.append(mk_out_dma(j, wout, NF // 2, NF))
                return post
            pending["post"] = mk_post()
        # chunk j+1's attention reads qt/kt/vt(j+1): those projections must
        # be fully emitted before the next emit_pass
        while proj_fillers:
            proj_fillers.pop(0)()
    if pending["pv"] is not None:
        pending["pv"]()
        pending["pv"] = None
    if pending["post"] is not None:
        pending["post"]()
        pending["post"] = None
    while proj_fillers:
        proj_fillers.pop(0)()
    while wo_fillers:
        wo_fillers.pop(0)()


def build_program():
    import concourse.tile as tile
    from concourse import bacc, mybir
    from contextlib import ExitStack

    F8 = mybir.dt.float8e4
    F16 = mybir.dt.float16
    F32 = mybir.dt.float32

    nc = bacc.Bacc("TRN2", target_bir_lowering=False, debug=False)
    xdt = F8 if SPLIT_PROJ else F16
    nsel = 2 if SPLIT_PROJ else 1
    wdt = F8 if SPLIT_PROJ else F16
    io = {
        "xq": nc.dram_tensor("xq", [P, NF, nsel, S], xdt,
                             kind="ExternalInput").ap(),
        "xk": nc.dram_tensor("xk", [P, NF, nsel, S], xdt,
                             kind="ExternalInput").ap(),
        "xv": nc.dram_tensor("xv", [P, NF, nsel, S], xdt,
                             kind="ExternalInput").ap(),
        "wq": nc.dram_tensor("wq", [P, NF, nsel, DKC], wdt,
                             kind="ExternalInput").ap(),
        "wk": nc.dram_tensor("wk", [P, NF, nsel, DKC], wdt,
                             kind="ExternalInput").ap(),
        "wv": nc.dram_tensor("wv", [P, NF, nsel, DKC], wdt,
                             kind="ExternalInput").ap(),
        "wo": nc.dram_tensor("wo", [P, 2, NF, P], F16,
                             kind="ExternalInput").ap(),
        "tri": nc.dram_tensor("tri", [P, P], F16, kind="ExternalInput").ap(),
        "eye": nc.dram_tensor("eye", [P, P], F16, kind="ExternalInput").ap(),
        "bq": nc.dram_tensor("bq", [P, 2], F32, kind="ExternalInput").ap(),
        "bk": nc.dram_tensor("bk", [P, 2], F32, kind="ExternalInput").ap(),
        "out_t": nc.dram_tensor("out_t", [D, S], F16,
                                kind="ExternalOutput").ap(),
    }
    if DEBUG:
        io["dbg_qt"] = nc.dram_tensor("dbg_qt", [P, 2, QC], F16,
                                      kind="ExternalOutput").ap()
        io["dbg_kt"] = nc.dram_tensor("dbg_kt", [P, 2, QC], F16,
                                      kind="ExternalOutput").ap()
        io["dbg_vt"] = nc.dram_tensor("dbg_vt", [P, HPC, 65], F16,
                                      kind="ExternalOutput").ap()
        io["dbg_pt"] = nc.dram_tensor("dbg_pt", [P, 2 * QC], F16,
                                      kind="ExternalOutput").ap()
        io["dbg_ctxs"] = nc.dram_tensor("dbg_ctxs", [P, 2, KPC, 64], F16,
                                        kind="ExternalOutput").ap()
    with tile.TileContext(nc) as tc, ExitStack() as ctx:
        _mha_body(ctx, tc, io)
    nc.compile()
    return nc


# ---------------------------------------------------------------------------
# Host side
# ---------------------------------------------------------------------------

def _np_reference(query, key, value, mask, Wq, bq, Wk, bk, Wv, bv, Wo, bo):
    q = (query.reshape(-1, D) @ Wq + bq).reshape(B, S, H, DK).transpose(0, 2, 1, 3)
    k = (key.reshape(-1, D) @ Wk + bk).reshape(B, S, H, DK).transpose(0, 2, 1, 3)
    v = (value.reshape(-1, D) @ Wv + bv).reshape(B, S, H, DK).transpose(0, 2, 1, 3)
    scores = np.einsum("bhqd,bhkd->bhqk", q, k) / math.sqrt(DK)
    scores = np.where(mask[:, None, :, :] == 0, np.float32(-1e9), scores)
    scores -= scores.max(axis=-1, keepdims=True)
    p = np.exp(scores)
    p /= p.sum(axis=-1, keepdims=True)
    x = np.einsum("bhqk,bhkd->bhqd", p, v)
    x = x.transpose(0, 2, 1, 3).reshape(B, -1, D)
    return (x @ Wo + bo).astype(np.float32)


def _split8(x):
    """fp32 -> (lo, hi) fp8e4 with hi = fp8(x), lo = fp8(x - hi)."""
    hi = x.astype(F8NP)
    lo = (x - hi.astype(np.float32)).astype(F8NP)
    return lo, hi


def _xlayout(x):
    """[S, D] batch slice -> [128, NF, nsel, S]: [p, f, sel, t] with
    sel = (lo, hi) fp8 split (or a single fp16 plane)."""
    xt = np.ascontiguousarray(x.T)  # [D, S]
    xt = xt.reshape(NF, P, S).transpose(1, 0, 2)  # [p, f, t]
    if not SPLIT_PROJ:
        return np.ascontiguousarray(xt[:, :, None, :]).astype(np.float16)
    lo, hi = _split8(xt.astype(np.float32))
    out = np.empty((P, NF, 2, S), F8NP)
    out[:, :, 0, :] = lo
    out[:, :, 1, :] = hi
    return out


def _wlayout(w):
    """[D, DKC] weight slice -> [128, NF, nsel, DKC]: [p, f, sel, m] with
    sel = (hi, lo) of the x32-scaled weight (or single fp16 plane)."""
    wr = w.reshape(NF, P, DKC).transpose(1, 0, 2)  # [p, f, m]
    if not SPLIT_PROJ:
        return np.ascontiguousarray(wr[:, :, None, :]).astype(np.float16)
    lo, hi = _split8(wr.astype(np.float32) * WSCALE)
    out = np.empty((P, NF, 2, DKC), F8NP)
    out[:, :, 0, :] = hi
    out[:, :, 1, :] = lo
    return out


def _shard_inputs(query, key, value, Wq, bq, Wk, bk, Wv, Wo):
    idx = np.arange(P)
    tri = (idx[:, None] <= idx[None, :]).astype(np.float16)  # [k, q] k<=q
    eye = np.eye(P, dtype=np.float16)
    in_maps = []
    xcache = {}
    for c in range(NCORES):
        bb, g = c // 4, c % 4
        sl = slice(g * DKC, (g + 1) * DKC)
        if bb not in xcache:
            xcache[bb] = {
                "xq": _xlayout(query[bb]),
                "xk": _xlayout(key[bb]),
                "xv": _xlayout(value[bb]),
            }
        wo = Wo[sl, :].reshape(2, P, NF, P).transpose(1, 0, 2, 3)
        in_maps.append({
            **xcache[bb],
            "wq": _wlayout(Wq[:, sl]),
            "wk": _wlayout(Wk[:, sl]),
            "wv": _wlayout(Wv[:, sl]),
            "wo": np.ascontiguousarray(wo).astype(np.float16),
            "tri": tri,
            "eye": eye,
            "bq": np.ascontiguousarray(bq[sl].reshape(2, P).T).astype(np.float32),
            "bk": np.ascontiguousarray(bk[sl].reshape(2, P).T).astype(np.float32),
        })
    return in_maps


def kernel(**inputs):
    query = np.asarray(inputs["query"], np.float32)
    key = np.asarray(inputs["key"], np.float32)
    value = np.asarray(inputs["value"], np.float32)
    mask = np.asarray(inputs["mask"])
    Wq = np.asarray(inputs["Wq"], np.float32)
    bq = np.asarray(inputs["bq"], np.float32)
    Wk = np.asarray(inputs["Wk"], np.float32)
    bk = np.asarray(inputs["bk"], np.float32)
    Wv = np.asarray(inputs["Wv"], np.float32)
    bv = np.asarray(inputs["bv"], np.float32)
    Wo = np.asarray(inputs["Wo"], np.float32)
    bo = np.asarray(inputs["bo"], np.float32)

    tril = np.tril(np.ones((S, S), np.int8))
    if mask.shape != (B, S, S) or not np.array_equal(
            (mask != 0).astype(np.int8), np.broadcast_to(tril, (B, S, S))):
        return _np_reference(query, key, value, mask,
                             Wq, bq, Wk, bk, Wv, bv, Wo, bo)

    in_maps = _shard_inputs(query, key, value, Wq, bq, Wk, bk, Wv, Wo)
    outs = _run_spmd(in_maps)  # [8, D, S]

    outs = outs.astype(np.float32)
    corr = (bv @ Wo + bo)[None, :]
    res = np.empty((B, S, D), np.float32)
    for bb in range(B):
        acc = outs[bb * 4:(bb + 1) * 4].sum(axis=0)  # [D, S]
        res[bb] = acc.T + corr
    return res


def _get_exec():
    if "exec" in _PROGRAM_CACHE:
        return _PROGRAM_CACHE["exec"]
    import jax
    from jax.sharding import Mesh, PartitionSpec
    from jax.experimental.shard_map import shard_map
    import concourse.mybir as mybir
    from concourse import bass2jax

    nc = build_program()
    _PROGRAM_CACHE["nc"] = nc
    bass2jax.install_neuronx_cc_hook()
    partition_name = nc.partition_id_tensor.name if nc.partition_id_tensor else None
    in_names, out_names, out_avals, zero_outs = [], [], [], []
    for alloc in nc.m.functions[0].allocations:
        if not isinstance(alloc, mybir.MemoryLocationSet):
            continue
        name = alloc.memorylocations[0].name
        if alloc.kind == "ExternalInput":
            if name != partition_name:
                in_names.append(name)
        elif alloc.kind == "ExternalOutput":
            out_names.append(name)
            shape = tuple(alloc.tensor_shape)
            dtype = mybir.dt.np(alloc.dtype)
            out_avals.append(jax.core.ShapedArray(shape, dtype))
            zero_outs.append(np.zeros(shape, dtype))
    n_params = len(in_names)
    all_in_names = list(in_names) + list(out_names)
    if partition_name is not None:
        all_in_names.append(partition_name)

    def _body(*args):
        operands = list(args)
        if partition_name is not None:
            operands.append(bass2jax.partition_id_tensor())
        return tuple(bass2jax._bass_exec_p.bind(
            *operands,
            out_avals=tuple(out_avals),
            in_names=tuple(all_in_names),
            out_names=tuple(out_names),
            lowering_input_output_aliases=(),
            sim_require_finite=True,
            sim_require_nnan=True,
            nc=nc,
        ))

    devices = jax.devices()[:NCORES]
    assert len(devices) >= NCORES, f"need {NCORES} cores, have {len(devices)}"
    mesh = Mesh(np.asarray(devices[:NCORES]), ("core",))
    fn = jax.jit(
        shard_map(_body, mesh=mesh,
                  in_specs=(PartitionSpec("core"),) * (n_params + len(zero_outs)),
                  out_specs=(PartitionSpec("core"),) * len(out_names),
                  check_rep=False),
        donate_argnums=tuple(range(n_params, n_params + len(out_names))),
        keep_unused=True)
    _PROGRAM_CACHE["exec"] = (fn, in_names, zero_outs)
    return _PROGRAM_CACHE["exec"]


def _run_spmd(in_maps):
    fn, in_names, zero_outs = _get_exec()
    concat_in = [np.concatenate([np.asarray(in_maps[c][nm])
                                 for c in range(NCORES)], axis=0)
                 for nm in in_names]
    concat_zero = [np.zeros((NCORES * z.shape[0], *z.shape[1:]), z.dtype)
                   for z in zero_outs]
    out = fn(*concat_in, *concat_zero)
    LAST["out"] = out
    return np.asarray(out[0]).reshape(NCORES, D, S)


# revision 5
# speedup vs baseline: 1.4791x; 1.0032x over previous
"""Trainium2 Bass kernel for 16-head causal multi-head attention (v2).

Problem: B=2, S=2048, D=1024, H=16 (head dim 64), causal mask.
    out = softmax((XqWq+bq)(XkWk+bk)^T / 8, causal) (XvWv+bv) Wo + bo

Sharding: 8 cores = 2 batches x 4 head-groups. Core c owns batch c//4 and
heads 4*(c%4)..4*(c%4)+3 (dk dims 256): Wq/Wk/Wv column-sliced, Wo
row-sliced. Each core computes a partial y^T [1024, 2048] for its batch;
host sums the 4 partials per batch and adds (bv @ Wo + bo).

Device-side design (per core):
  - Projections in fp8e4 DoubleRow with residual hi/lo split:
    q = X_hi@W_hi + (X_lo@W_hi + X_hi@W_lo), W pre-scaled by 32 so its fp8
    values are unit-variance; the 1/32 is folded into the PSUM eviction.
    Cost model charge: 0.75x of fp16, at ~2e-3 end-to-end error.
  - QK in fp16, scores transposed S^T[k, q] so exp output P^T is ready as
    the PV stationary operand.
  - PV "swapped": out ctx[q, dk+1] with P^T [k, q-subtile] stationary and
    V-augmented [k, 65] moving -> charge 65 rows per (ktile, qsub, head)
    instead of 512 (2x PE saving); the ones column (x32) yields softmax
    sums so normalization is a per-partition (per-token) scalar multiply.
  - ctx is transposed back (PE is_transpose matmuls into one psum bank)
    for the fp16 Wo matmuls; Wo/transpose work of chunk j is emitted as
    PE filler inside chunk j+1's ACT-bound attention loop.
  - 4 heads are processed as 2 passes of 2 heads to fit PSUM:
    banks = scores dbuf (2x2) + PV accum (2) + aux dbuf (2).
"""

import math

import numpy as np
import ml_dtypes

# Full-problem constants
B, S, D, H = 2, 2048, 1024, 16
DK = D // H  # 64
NCORES = 8
HPC = 4          # heads per core
DKC = HPC * DK   # 256 dk dims per core
P = 128
QC = 512         # tokens per chunk
NCH = S // QC    # 4 chunks
KPC = QC // P    # 4 k-tiles per chunk
NF = D // P      # 8 feature tiles
WSCALE = 32.0    # fp8 weight pre-scale

SPLIT_PROJ = True  # fp8 DoubleRow split projections (False -> fp16 proj)
DEBUG = False      # add debug dram dumps of intermediates

_PROGRAM_CACHE = {}
LAST = {}

F8NP = ml_dtypes.float8_e4m3


# ---------------------------------------------------------------------------
# Device program
# ---------------------------------------------------------------------------

def _mha_body(ctx, tc, io):
    from concourse import mybir

    F8 = mybir.dt.float8e4
    F16 = mybir.dt.float16
    F32 = mybir.dt.float32
    Exp = mybir.ActivationFunctionType.Exp
    DR = mybir.MatmulPerfMode.DoubleRow
    MUL = mybir.AluOpType.mult
    ADD = mybir.AluOpType.add

    nc = tc.nc

    consts = ctx.enter_context(tc.tile_pool(name="consts", bufs=1))
    xs = ctx.enter_context(tc.tile_pool(name="xs", bufs=1))
    persist = ctx.enter_context(tc.tile_pool(name="persist", bufs=1))
    pts = ctx.enter_context(tc.tile_pool(name="pts", bufs=4))
    wouts = ctx.enter_context(tc.tile_pool(name="wouts", bufs=4))
    pspool = ctx.enter_context(tc.tile_pool(name="ps", bufs=1, space="PSUM"))

    # ---- constants & inputs (DMA order = first-use order) ----------------
    # chunk-0 prologue data first (wq/wk/wv + first-chunk x), tiny biases,
    # then the rest of x, then late-use constants (tri/eye/wo).
    if SPLIT_PROJ:
        wdt, wshape = F8, [P, NF, 2, DKC]
    else:
        wdt, wshape = F16, [P, NF, 1, DKC]
    xdt = F8 if SPLIT_PROJ else F16
    nsel = 2 if SPLIT_PROJ else 1

    bx0, bxr = {}, {}
    bq_sb = consts.tile([P, 2], F32, tag="bq", name="bq_sb")
    nc.sync.dma_start(bq_sb[:], io["bq"])
    bk_sb = consts.tile([P, 2], F32, tag="bk", name="bk_sb")
    nc.sync.dma_start(bk_sb[:], io["bk"])
    w_sbs = {}
    for wnm, xnm in (("wq", "q"), ("wk", "k"), ("wv", "v")):
        w_sbs[wnm] = consts.tile(wshape, wdt, tag=wnm, name=wnm + "_sb")
        nc.sync.dma_start(w_sbs[wnm][:], io[wnm])
        t0 = xs.tile([P, NF, nsel, QC], xdt, tag=f"x{xnm}0", name=f"x{xnm}0")
        if xnm == "k":
            nc.sync.dma_start(t0[:, 0:NF // 2], io["xk"][:, 0:NF // 2, :, 0:QC])
            nc.sync.dma_start(t0[:, NF // 2:], io["xk"][:, NF // 2:, :, 0:QC])
        else:
            nc.sync.dma_start(t0[:], io["x" + xnm][:, :, :, 0:QC])
        bx0[xnm] = t0
    wq_sb, wk_sb, wv_sb = w_sbs["wq"], w_sbs["wk"], w_sbs["wv"]
    tri_sb = consts.tile([P, P], F16, tag="tri", name="tri_sb")
    nc.sync.dma_start(tri_sb[:], io["tri"])
    eye_sb = consts.tile([P, P], F16, tag="eye", name="eye_sb")
    nc.sync.dma_start(eye_sb[:], io["eye"])
    for nm in ("q", "k", "v"):
        bxr[nm] = xs.tile([P, NF, nsel, S - QC], xdt, tag=f"x{nm}r",
                          name=f"x{nm}r")
    # per-chunk input DMAs, interleaved q/k/v, in consumption order
    for j in range(1, NCH):
        for nm in ("q", "k", "v"):
            nc.sync.dma_start(
                bxr[nm][:, :, :, (j - 1) * QC:j * QC],
                io["x" + nm][:, :, :, j * QC:(j + 1) * QC])
    wo_sb = consts.tile([P, 2, NF, P], F16, tag="wo", name="wo_sb")
    nc.sync.dma_start(wo_sb[:], io["wo"])

    # PE p-state warmup: the startup is DMA-bound; dependency-free dummy
    # matmuls keep the tensor engine clock ramped so the first real
    # projections run at full speed instead of the mid-p-state 2x penalty.
    warm = consts.tile([P, QC], F16, tag="warm", name="warm")
    nc.vector.memset(warm[:], 0.0)
    wps = pspool.tile([P, QC], F32, tag="aux", name="warmps", bufs=2)
    for _ in range(20):
        nc.tensor.matmul(wps[:], warm[:, 0:P], warm[:],
                         start=True, stop=True, skip_group_check=True)

    def xsl(nm, lo, hi):
        """[P, NF, nsel, hi-lo] view of input nm tokens [lo:hi)."""
        if hi <= QC:
            return bx0[nm][:, :, :, lo:hi]
        return bxr[nm][:, :, :, lo - QC:hi - QC]

    def ps_tile(tag, shape=None, dtype=F32, bufs=None):
        shape = shape or [P, QC]
        return pspool.tile(shape, dtype, tag=tag, name=tag, bufs=bufs)

    # ---- projection units (queued as PE fillers) -------------------------
    def mk_proj_qk_unit(j, w_sb, b_sb, out, t, srcnm):
        """One psum tile of a Q/K projection: matmuls + biased fp16 evict."""
        co = j * QC

        def go():
            pp = ps_tile("aux", bufs=2)
            if SPLIT_PROJ:
                first = True
                for hh in range(2):  # token halves (moving free <= 512)
                    ts0, ts1 = co + hh * 256, co + (hh + 1) * 256
                    for f in range(0, NF, 2):  # main terms: hi x hi pairs
                        nc.tensor.matmul(
                            pp[:, hh * 256:(hh + 1) * 256],
                            w_sb[:, f:f + 2, 0, t * P:(t + 1) * P],
                            xsl(srcnm, ts0, ts1)[:, f:f + 2, 1, :],
                            start=first, stop=False, perf_mode=DR,
                            skip_group_check=True)
                        first = False
                    for f in range(NF):  # cross terms: (hi,lo)x(lo,hi)
                        nc.tensor.matmul(
                            pp[:, hh * 256:(hh + 1) * 256],
                            w_sb[:, f, 0:2, t * P:(t + 1) * P],
                            xsl(srcnm, ts0, ts1)[:, f, 0:2, :],
                            start=False, stop=(f == NF - 1 and hh == 1),
                            perf_mode=DR, skip_group_check=True)
                scl = 1.0 / WSCALE
            else:
                for f in range(NF):
                    nc.tensor.matmul(
                        pp[:], w_sb[:, f, 0, t * P:(t + 1) * P],
                        xsl(srcnm, co, co + QC)[:, f, 0, :],
                        start=(f == 0), stop=(f == NF - 1))
                scl = 1.0
            nc.vector.tensor_scalar(out[:, t, :], pp[:], scl,
                                    b_sb[:, t:t + 1], MUL, ADD)
        return go

    def mk_proj_v_unit(j, t4):
        """One 128-token tile of the V projection -> vt tile (token-major)."""
        co = j * QC

        def go():
            pp = ps_tile("aux", [P, DKC], bufs=2)
            ts0, ts1 = co + t4 * P, co + (t4 + 1) * P
            if SPLIT_PROJ:
                first = True
                for f in range(0, NF, 2):
                    nc.tensor.matmul(
                        pp[:], xsl("v", ts0, ts1)[:, f:f + 2, 1, :],
                        wv_sb[:, f:f + 2, 0, :],
                        start=first, stop=False, perf_mode=DR,
                        skip_group_check=True)
                    first = False
                for f in range(NF):
                    nc.tensor.matmul(
                        pp[:], xsl("v", ts0, ts1)[:, f, 0:2, :],
                        wv_sb[:, f, 0:2, :],
                        start=False, stop=(f == NF - 1),
                        perf_mode=DR, skip_group_check=True)
                scl = 1.0 / WSCALE
            else:
                for f in range(NF):
                    nc.tensor.matmul(
                        pp[:], xsl("v", ts0, ts1)[:, f, 0, :],
                        wv_sb[:, f, 0, :],
                        start=(f == 0), stop=(f == NF - 1))
                scl = 1.0
            kt = j * KPC + t4
            vt = vt_tiles[kt]
            # ones column: PV's column 64 accumulates the softmax sums
            nc.vector.memset(vt[:, :, 64:65], 1.0)
            nc.vector.tensor_scalar_mul(
                vt[:, :, 0:64], pp[:].rearrange("p (h d) -> p h d", d=64), scl)
        return go

    qt = {}
    kt_tiles = {}
    vt_tiles = {}
    # cross-segment software pipeline state: the previous event's PV
    # emission and the previous segment's normalize hook
    pending = {"pv": None, "post": None, "tick": 0}

    # Filler queues of PE closures drained inside ACT-bound attention
    # k-loops: `proj_fillers` (next chunk's projections + ctx transposes —
    # must drain promptly) and `wo_fillers` (Wo matmuls + output DMA of
    # finished chunks — deferred into the LAST chunk's long k-loop, which
    # is the only ACT-bound phase with spare PE time).
    proj_fillers = []
    wo_fillers = []

    def drain_filler(n=1, wo_ok=False):
        for _ in range(n):
            if proj_fillers:
                proj_fillers.pop(0)()
            elif wo_ok and wo_fillers:
                wo_fillers.pop(0)()

    def queue_proj_qk(jn):
        for (w_sb, b_sb, tag, srcnm) in ((wq_sb, bq_sb, f"qt{jn % 2}", "q"),
                                         (wk_sb, bk_sb, f"kt{jn}", "k")):
            out = persist.tile([P, 2, QC], F16, tag=tag, name=tag)
            if srcnm == "q":
                qt[jn % 2] = out
            else:
                kt_tiles[jn] = out
            for t in range(2):
                proj_fillers.append(
                    mk_proj_qk_unit(jn, w_sb, b_sb, out, t, srcnm))

    def queue_proj_v(jn):
        for t4 in range(KPC):
            kt = jn * KPC + t4
            vt_tiles[kt] = persist.tile([P, HPC, 65], F16, tag=f"vt{kt}",
                                        name=f"vt{kt}")
            proj_fillers.append(mk_proj_v_unit(jn, t4))

    def queue_proj(jn):
        queue_proj_qk(jn)
        queue_proj_v(jn)

    # ---- attention pass (2 heads) ---------------------------------------
    def emit_pass(j, p):
        """Attention for chunk j, heads {2p, 2p+1}. Returns pvacc tiles."""
        nkt = KPC * (j + 1)
        qtile = qt[j % 2]
        pv = [ps_tile(f"pv{l}", [P, KPC, 80]) for l in range(2)]

        def head_slices(l):
            """(dim-tile t, partition offset) for local head l of pass p."""
            hh = 2 * p + l
            return hh // 2, (hh % 2) * 64

        def emit_qk_exp(kt):
            jk = kt // KPC          # source chunk of this k-tile
            ko = (kt % KPC) * P
            tdiag = kt - KPC * j    # >=0 on the diagonal chunk
            ktile = kt_tiles[jk]
            sw = ps_tile("swA" if kt % 2 == 0 else "swB", [P, 2 * QC])
            c0 = max(tdiag, 0) * P
            for l in range(2):
                t, po = head_slices(l)
                nc.tensor.matmul(
                    sw[:, l * QC + c0:(l + 1) * QC],
                    ktile[po:po + 64, t, ko:ko + P],
                    qtile[po:po + 64, t, c0:QC],
                    start=True, stop=True)
            pt = pts.tile([P, 2 * QC], F16, tag="pt", name="pt")
            if c0:
                # one strided exp covering both heads' live columns
                swv = sw[:].rearrange("p (l q) -> p l q", l=2)[:, :, c0:QC]
                ptv = pt[:].rearrange("p (l q) -> p l q", l=2)[:, :, c0:QC]
                nc.scalar.activation(ptv, swv, Exp, scale=0.125)
            else:
                nc.scalar.activation(pt[:], sw[:], Exp, scale=0.125)
            if tdiag >= 0:
                # masked-diagonal multiply on the (otherwise idle) Pool
                # engine so exp-dependent work never blocks DVE's evictions
                for l in range(2):
                    nc.gpsimd.tensor_mul(pt[:, l * QC + c0:l * QC + c0 + P],
                                         pt[:, l * QC + c0:l * QC + c0 + P],
                                         tri_sb[:])
            if DEBUG and j == 0 and p == 0 and kt == 0:
                nc.sync.dma_start(io["dbg_pt"], pt[:])

            def emit_pv():
                vt = vt_tiles[kt]
                for l in range(2):
                    hh = 2 * p + l
                    for t in range(max(tdiag, 0), KPC):
                        nc.tensor.matmul(
                            pv[l][:, t, 0:65],
                            pt[:, l * QC + t * P:l * QC + (t + 1) * P],
                            vt[:, hh, :],
                            start=(kt == 0 and t == 0),
                            stop=(kt == KPC * j + t),
                            skip_group_check=True)
            return emit_pv

        # Globally software-pipelined: the PV of the previous event — which
        # may belong to the PREVIOUS segment — is emitted after this event's
        # QK+exp, so the ACT engine never waits at segment boundaries. The
        # previous segment's normalize hook runs right after its last PV.
        wo_ok = (j >= NCH - 2)
        for kt in range(nkt):
            pv_next = emit_qk_exp(kt)
            if pending["pv"] is not None:
                pending["pv"]()
            if pending["post"] is not None:
                pending["post"]()
                pending["post"] = None
            drain_filler(2 if len(proj_fillers) > 8 else 1, wo_ok)
            pending["pv"] = pv_next
        return pv

    # ---- normalize + transpose + Wo for one chunk ------------------------
    def emit_norm(j, p, pv):
        """Normalize pass p of chunk j into ctx_sb (DVE), return ctx_sb."""
        cs = persist.tile([P, 2, KPC, 64], F16, tag=f"ctxs{p}",
                          name=f"ctxs{p}")
        for l in range(2):
            rc = persist.tile([P, KPC], F32, tag=f"rc{p}{l}", name=f"rc{p}{l}")
            nc.vector.reciprocal(rc[:], pv[l][:, :, 64:65].squeeze(2))
            for t in range(KPC):
                nc.vector.tensor_scalar_mul(cs[:, l, t, :], pv[l][:, t, 0:64],
                                            rc[:, t:t + 1])
        return cs

    def mk_transp(cs, ctx_t, p):
        def go():
            trp = ps_tile("aux", [P, QC], F16, bufs=2)
            for l in range(2):
                for t in range(KPC):
                    nc.tensor.matmul(
                        trp[l * 64:(l + 1) * 64, t * P:(t + 1) * P],
                        cs[:, l, t, :], eye_sb[:],
                        is_transpose=True,
                        start=(t == 0), stop=(t == KPC - 1),
                        skip_group_check=True,
                        tile_position=(0, l * 64))
            nc.vector.tensor_copy(ctx_t[:, p, :], trp[:])
        return go

    def mk_wo(ctx_t, wout, m, act_ok, t_only=None):
        def go():
            if act_ok:
                # tail of the last chunk: score banks are free, use them to
                # deepen the psum rotation so Wo/evict fully pipeline
                tag = ("aux", "swA", "swB")[m % 3]
            else:
                tag = "aux"
            po = ps_tile(tag, [P, QC], bufs=2 if tag == "aux" else None)
            ts = (0, 1) if t_only is None else (t_only,)
            for i, t in enumerate(ts):
                nc.tensor.matmul(po[:], wo_sb[:, t, m, :], ctx_t[:, t, :],
                                 start=(i == 0), stop=(i == len(ts) - 1))
            if act_ok and m % 2:
                # tail chunk: ACT is idle there, share the eviction load
                nc.scalar.copy(wout[:, m, :], po[:])
            else:
                nc.vector.tensor_copy(wout[:, m, :], po[:])
        return go

    def mk_out_dma(j, wout, m0, m1, dst="out_t"):
        def go():
            nc.gpsimd.dma_start(
                io[dst].rearrange("(m p) s -> p m s", p=P)
                [:, m0:m1, j * QC:(j + 1) * QC], wout[:, m0:m1, :])
        return go

    # ---- main loop -------------------------------------------------------
    # chunk 0's projections run up front; chunk j+1's are drained as PE
    # fillers inside chunk j's ACT-bound attention, together with chunk
    # j-1/j's transpose + Wo + output-DMA epilogue.
    queue_proj_qk(0)
    while proj_fillers:
        proj_fillers.pop(0)()
    queue_proj_v(0)
    for j in range(NCH):
        if j + 1 < NCH:
            queue_proj(j + 1)
        ctx_t = persist.tile([P, 2, QC], F16, tag=f"ctxt{j}",
                             name=f"ctxt{j}")
        wout = wouts.tile([P, NF, QC], F16, tag="wout", name="wout")
        for p in range(2):
            pv = emit_pass(j, p)

            def mk_post(j=j, p=p, pv=pv, ctx_t=ctx_t, wout=wout):
                last = (j == NCH - 1)

                def post():
                    cs = emit_norm(j, p, pv)
                    proj_fillers.append(mk_transp(cs, ctx_t, p))
                    if last and p == 0:
                        # last chunk: pass-0 half of Wo runs during pass 1's
                        # k-loop, into a second output summed on the host
                        w2 = wouts.tile([P, NF, QC], F16, tag="wout",
                                        name="wout2")
                        for m in range(NF):
                            wo_fillers.append(
                                mk_wo(ctx_t, w2, m, False, t_only=0))
                            if m == NF // 2 - 1:
                                wo_fillers.append(
                                    mk_out_dma(j, w2, 0, NF // 2, "out_t2"))
                        wo_fillers.append(
                            mk_out_dma(j, w2, NF // 2, NF, "out_t2"))
                    elif p == 1:
                        t_only = 1 if last else None
                        for m in range(NF):
                            wo_fillers.append(
                                mk_wo(ctx_t, wout, m, last, t_only=t_only))
                            if m == NF // 2 - 1:
                                wo_fillers.append(
                                    mk_out_dma(j, wout, 0, NF // 2))
                        wo_fillers.append(mk_out_dma(j, wout, NF // 2, NF))
                return post
            pending["post"] = mk_post()
        # chunk j+1's attention reads qt/kt/vt(j+1): those projections must
        # be fully emitted before the next emit_pass
        while proj_fillers:
            proj_fillers.pop(0)()
    if pending["pv"] is not None:
        pending["pv"]()
        pending["pv"] = None
    if pending["post"] is not None:
        pending["post"]()
        pending["post"] = None
    while proj_fillers:
        proj_fillers.pop(0)()
    while wo_fillers:
        wo_fillers.pop(0)()


def build_program():
    import concourse.tile as tile
    from concourse import bacc, mybir
    from contextlib import ExitStack

    F8 = mybir.dt.float8e4
    F16 = mybir.dt.float16
    F32 = mybir.dt.float32

    nc = bacc.Bacc("TRN2", target_bir_lowering=False, debug=False)
    xdt = F8 if SPLIT_PROJ else F16
    nsel = 2 if SPLIT_PROJ else 1
    wdt = F8 if SPLIT_PROJ else F16
    io = {
        "xq": nc.dram_tensor("xq", [P, NF, nsel, S], xdt,
                             kind="ExternalInput").ap(),
        "xk": nc.dram_tensor("xk", [P, NF, nsel, S], xdt,
                             kind="ExternalInput").ap(),
        "xv": nc.dram_tensor("xv", [P, NF, nsel, S], xdt,
                             kind="ExternalInput").ap(),
        "wq": nc.dram_tensor("wq", [P, NF, nsel, DKC], wdt,
                             kind="ExternalInput").ap(),
        "wk": nc.dram_tensor("wk", [P, NF, nsel, DKC], wdt,
                             kind="ExternalInput").ap(),
        "wv": nc.dram_tensor("wv", [P, NF, nsel, DKC], wdt,
                             kind="ExternalInput").ap(),
        "wo": nc.dram_tensor("wo", [P, 2, NF, P], F16,
                             kind="ExternalInput").ap(),
        "tri": nc.dram_tensor("tri", [P, P], F16, kind="ExternalInput").ap(),
        "eye": nc.dram_tensor("eye", [P, P], F16, kind="ExternalInput").ap(),
        "bq": nc.dram_tensor("bq", [P, 2], F32, kind="ExternalInput").ap(),
        "bk": nc.dram_tensor("bk", [P, 2], F32, kind="ExternalInput").ap(),
        "out_t": nc.dram_tensor("out_t", [D, S], F16,
                                kind="ExternalOutput").ap(),
        "out_t2": nc.dram_tensor("out_t2", [D, S], F16,
                                 kind="ExternalOutput").ap(),
    }
    if DEBUG:
        io["dbg_qt"] = nc.dram_tensor("dbg_qt", [P, 2, QC], F16,
                                      kind="ExternalOutput").ap()
        io["dbg_kt"] = nc.dram_tensor("dbg_kt", [P, 2, QC], F16,
                                      kind="ExternalOutput").ap()
        io["dbg_vt"] = nc.dram_tensor("dbg_vt", [P, HPC, 65], F16,
                                      kind="ExternalOutput").ap()
        io["dbg_pt"] = nc.dram_tensor("dbg_pt", [P, 2 * QC], F16,
                                      kind="ExternalOutput").ap()
        io["dbg_ctxs"] = nc.dram_tensor("dbg_ctxs", [P, 2, KPC, 64], F16,
                                        kind="ExternalOutput").ap()
    with tile.TileContext(nc) as tc, ExitStack() as ctx:
        _mha_body(ctx, tc, io)
    nc.compile()
    return nc


# ---------------------------------------------------------------------------
# Host side
# ---------------------------------------------------------------------------

def _np_reference(query, key, value, mask, Wq, bq, Wk, bk, Wv, bv, Wo, bo):
    q = (query.reshape(-1, D) @ Wq + bq).reshape(B, S, H, DK).transpose(0, 2, 1, 3)
    k = (key.reshape(-1, D) @ Wk + bk).reshape(B, S, H, DK).transpose(0, 2, 1, 3)
    v = (value.reshape(-1, D) @ Wv + bv).reshape(B, S, H, DK).transpose(0, 2, 1, 3)
    scores = np.einsum("bhqd,bhkd->bhqk", q, k) / math.sqrt(DK)
    scores = np.where(mask[:, None, :, :] == 0, np.float32(-1e9), scores)
    scores -= scores.max(axis=-1, keepdims=True)
    p = np.exp(scores)
    p /= p.sum(axis=-1, keepdims=True)
    x = np.einsum("bhqk,bhkd->bhqd", p, v)
    x = x.transpose(0, 2, 1, 3).reshape(B, -1, D)
    return (x @ Wo + bo).astype(np.float32)


def _split8(x):
    """fp32 -> (lo, hi) fp8e4 with hi = fp8(x), lo = fp8(x - hi)."""
    hi = x.astype(F8NP)
    lo = (x - hi.astype(np.float32)).astype(F8NP)
    return lo, hi


def _xlayout(x):
    """[S, D] batch slice -> [128, NF, nsel, S]: [p, f, sel, t] with
    sel = (lo, hi) fp8 split (or a single fp16 plane)."""
    xt = np.ascontiguousarray(x.T)  # [D, S]
    xt = xt.reshape(NF, P, S).transpose(1, 0, 2)  # [p, f, t]
    if not SPLIT_PROJ:
        return np.ascontiguousarray(xt[:, :, None, :]).astype(np.float16)
    lo, hi = _split8(xt.astype(np.float32))
    out = np.empty((P, NF, 2, S), F8NP)
    out[:, :, 0, :] = lo
    out[:, :, 1, :] = hi
    return out


def _wlayout(w):
    """[D, DKC] weight slice -> [128, NF, nsel, DKC]: [p, f, sel, m] with
    sel = (hi, lo) of the x32-scaled weight (or single fp16 plane)."""
    wr = w.reshape(NF, P, DKC).transpose(1, 0, 2)  # [p, f, m]
    if not SPLIT_PROJ:
        return np.ascontiguousarray(wr[:, :, None, :]).astype(np.float16)
    lo, hi = _split8(wr.astype(np.float32) * WSCALE)
    out = np.empty((P, NF, 2, DKC), F8NP)
    out[:, :, 0, :] = hi
    out[:, :, 1, :] = lo
    return out


def _shard_inputs(query, key, value, Wq, bq, Wk, bk, Wv, Wo):
    idx = np.arange(P)
    tri = (idx[:, None] <= idx[None, :]).astype(np.float16)  # [k, q] k<=q
    eye = np.eye(P, dtype=np.float16)
    in_maps = []
    xcache = {}
    for c in range(NCORES):
        bb, g = c // 4, c % 4
        sl = slice(g * DKC, (g + 1) * DKC)
        if bb not in xcache:
            xcache[bb] = {
                "xq": _xlayout(query[bb]),
                "xk": _xlayout(key[bb]),
                "xv": _xlayout(value[bb]),
            }
        wo = Wo[sl, :].reshape(2, P, NF, P).transpose(1, 0, 2, 3)
        in_maps.append({
            **xcache[bb],
            "wq": _wlayout(Wq[:, sl]),
            "wk": _wlayout(Wk[:, sl]),
            "wv": _wlayout(Wv[:, sl]),
            "wo": np.ascontiguousarray(wo).astype(np.float16),
            "tri": tri,
            "eye": eye,
            "bq": np.ascontiguousarray(bq[sl].reshape(2, P).T).astype(np.float32),
            "bk": np.ascontiguousarray(bk[sl].reshape(2, P).T).astype(np.float32),
        })
    return in_maps


def kernel(**inputs):
    query = np.asarray(inputs["query"], np.float32)
    key = np.asarray(inputs["key"], np.float32)
    value = np.asarray(inputs["value"], np.float32)
    mask = np.asarray(inputs["mask"])
    Wq = np.asarray(inputs["Wq"], np.float32)
    bq = np.asarray(inputs["bq"], np.float32)
    Wk = np.asarray(inputs["Wk"], np.float32)
    bk = np.asarray(inputs["bk"], np.float32)
    Wv = np.asarray(inputs["Wv"], np.float32)
    bv = np.asarray(inputs["bv"], np.float32)
    Wo = np.asarray(inputs["Wo"], np.float32)
    bo = np.asarray(inputs["bo"], np.float32)

    tril = np.tril(np.ones((S, S), np.int8))
    if mask.shape != (B, S, S) or not np.array_equal(
            (mask != 0).astype(np.int8), np.broadcast_to(tril, (B, S, S))):
        return _np_reference(query, key, value, mask,
                             Wq, bq, Wk, bk, Wv, bv, Wo, bo)

    in_maps = _shard_inputs(query, key, value, Wq, bq, Wk, bk, Wv, Wo)
    outs = _run_spmd(in_maps)  # [8, D, S]

    outs = outs.astype(np.float32)
    corr = (bv @ Wo + bo)[None, :]
    res = np.empty((B, S, D), np.float32)
    for bb in range(B):
        acc = outs[bb * 4:(bb + 1) * 4].sum(axis=0)  # [D, S]
        res[bb] = acc.T + corr
    return res


def _get_exec():
    if "exec" in _PROGRAM_CACHE:
        return _PROGRAM_CACHE["exec"]
    import jax
    from jax.sharding import Mesh, PartitionSpec
    from jax.experimental.shard_map import shard_map
    import concourse.mybir as mybir
    from concourse import bass2jax

    nc = build_program()
    _PROGRAM_CACHE["nc"] = nc
    bass2jax.install_neuronx_cc_hook()
    partition_name = nc.partition_id_tensor.name if nc.partition_id_tensor else None
    in_names, out_names, out_avals, zero_outs = [], [], [], []
    for alloc in nc.m.functions[0].allocations:
        if not isinstance(alloc, mybir.MemoryLocationSet):
            continue
        name = alloc.memorylocations[0].name
        if alloc.kind == "ExternalInput":
            if name != partition_name:
                in_names.append(name)
        elif alloc.kind == "ExternalOutput":
            out_names.append(name)
            shape = tuple(alloc.tensor_shape)
            dtype = mybir.dt.np(alloc.dtype)
            out_avals.append(jax.core.ShapedArray(shape, dtype))
            zero_outs.append(np.zeros(shape, dtype))
    n_params = len(in_names)
    all_in_names = list(in_names) + list(out_names)
    if partition_name is not None:
        all_in_names.append(partition_name)

    def _body(*args):
        operands = list(args)
        if partition_name is not None:
            operands.append(bass2jax.partition_id_tensor())
        return tuple(bass2jax._bass_exec_p.bind(
            *operands,
            out_avals=tuple(out_avals),
            in_names=tuple(all_in_names),
            out_names=tuple(out_names),
            lowering_input_output_aliases=(),
            sim_require_finite=True,
            sim_require_nnan=True,
            nc=nc,
        ))

    devices = jax.devices()[:NCORES]
    assert len(devices) >= NCORES, f"need {NCORES} cores, have {len(devices)}"
    mesh = Mesh(np.asarray(devices[:NCORES]), ("core",))
    fn = jax.jit(
        shard_map(_body, mesh=mesh,
                  in_specs=(PartitionSpec("core"),) * (n_params + len(zero_outs)),
                  out_specs=(PartitionSpec("core"),) * len(out_names),
                  check_rep=False),
        donate_argnums=tuple(range(n_params, n_params + len(out_names))),
        keep_unused=True)
    _PROGRAM_CACHE["exec"] = (fn, in_names, zero_outs)
    return _PROGRAM_CACHE["exec"]


def _run_spmd(in_maps):
    fn, in_names, zero_outs = _get_exec()
    concat_in = [np.concatenate([np.asarray(in_maps[c][nm])
                                 for c in range(NCORES)], axis=0)
                 for nm in in_names]
    concat_zero = [np.zeros((NCORES * z.shape[0], *z.shape[1:]), z.dtype)
                   for z in zero_outs]
    out = fn(*concat_in, *concat_zero)
    LAST["out"] = out
    acc = np.asarray(out[0]).astype(np.float32)
    for o in out[1:]:
        acc = acc + np.asarray(o).astype(np.float32)
    return acc.reshape(NCORES, D, S)


# revision 6
# speedup vs baseline: 1.4805x; 1.0009x over previous
"""Trainium2 Bass kernel for 16-head causal multi-head attention (v2).

Problem: B=2, S=2048, D=1024, H=16 (head dim 64), causal mask.
    out = softmax((XqWq+bq)(XkWk+bk)^T / 8, causal) (XvWv+bv) Wo + bo

Sharding: 8 cores = 2 batches x 4 head-groups. Core c owns batch c//4 and
heads 4*(c%4)..4*(c%4)+3 (dk dims 256): Wq/Wk/Wv column-sliced, Wo
row-sliced. Each core computes a partial y^T [1024, 2048] for its batch;
host sums the 4 partials per batch and adds (bv @ Wo + bo).

Device-side design (per core):
  - Projections in fp8e4 DoubleRow with residual hi/lo split:
    q = X_hi@W_hi + (X_lo@W_hi + X_hi@W_lo), W pre-scaled by 32 so its fp8
    values are unit-variance; the 1/32 is folded into the PSUM eviction.
    Cost model charge: 0.75x of fp16, at ~2e-3 end-to-end error.
  - QK in fp16, scores transposed S^T[k, q] so exp output P^T is ready as
    the PV stationary operand.
  - PV "swapped": out ctx[q, dk+1] with P^T [k, q-subtile] stationary and
    V-augmented [k, 65] moving -> charge 65 rows per (ktile, qsub, head)
    instead of 512 (2x PE saving); the ones column (x32) yields softmax
    sums so normalization is a per-partition (per-token) scalar multiply.
  - ctx is transposed back (PE is_transpose matmuls into one psum bank)
    for the fp16 Wo matmuls; Wo/transpose work of chunk j is emitted as
    PE filler inside chunk j+1's ACT-bound attention loop.
  - 4 heads are processed as 2 passes of 2 heads to fit PSUM:
    banks = scores dbuf (2x2) + PV accum (2) + aux dbuf (2).
"""

import math

import numpy as np
import ml_dtypes

# Full-problem constants
B, S, D, H = 2, 2048, 1024, 16
DK = D // H  # 64
NCORES = 8
HPC = 4          # heads per core
DKC = HPC * DK   # 256 dk dims per core
P = 128
QC = 512         # tokens per chunk
NCH = S // QC    # 4 chunks
KPC = QC // P    # 4 k-tiles per chunk
NF = D // P      # 8 feature tiles
WSCALE = 32.0    # fp8 weight pre-scale

SPLIT_PROJ = True  # fp8 DoubleRow split projections (False -> fp16 proj)
DEBUG = False      # add debug dram dumps of intermediates

_PROGRAM_CACHE = {}
LAST = {}

F8NP = ml_dtypes.float8_e4m3


# ---------------------------------------------------------------------------
# Device program
# ---------------------------------------------------------------------------

def _mha_body(ctx, tc, io):
    from concourse import mybir

    F8 = mybir.dt.float8e4
    F16 = mybir.dt.float16
    F32 = mybir.dt.float32
    Exp = mybir.ActivationFunctionType.Exp
    DR = mybir.MatmulPerfMode.DoubleRow
    MUL = mybir.AluOpType.mult
    ADD = mybir.AluOpType.add

    nc = tc.nc

    consts = ctx.enter_context(tc.tile_pool(name="consts", bufs=1))
    xs = ctx.enter_context(tc.tile_pool(name="xs", bufs=1))
    persist = ctx.enter_context(tc.tile_pool(name="persist", bufs=1))
    pts = ctx.enter_context(tc.tile_pool(name="pts", bufs=6))
    wouts = ctx.enter_context(tc.tile_pool(name="wouts", bufs=6))
    pspool = ctx.enter_context(tc.tile_pool(name="ps", bufs=1, space="PSUM"))

    # ---- constants & inputs (DMA order = first-use order) ----------------
    # chunk-0 prologue data first (wq/wk/wv + first-chunk x), tiny biases,
    # then the rest of x, then late-use constants (tri/eye/wo).
    if SPLIT_PROJ:
        wdt, wshape = F8, [P, NF, 2, DKC]
    else:
        wdt, wshape = F16, [P, NF, 1, DKC]
    xdt = F8 if SPLIT_PROJ else F16
    nsel = 2 if SPLIT_PROJ else 1

    bx0, bxr = {}, {}
    bq_sb = consts.tile([P, 2], F32, tag="bq", name="bq_sb")
    nc.sync.dma_start(bq_sb[:], io["bq"])
    bk_sb = consts.tile([P, 2], F32, tag="bk", name="bk_sb")
    nc.sync.dma_start(bk_sb[:], io["bk"])
    w_sbs = {}
    for wnm, xnm in (("wq", "q"), ("wk", "k"), ("wv", "v")):
        w_sbs[wnm] = consts.tile(wshape, wdt, tag=wnm, name=wnm + "_sb")
        nc.sync.dma_start(w_sbs[wnm][:], io[wnm])
        t0 = xs.tile([P, NF, nsel, QC], xdt, tag=f"x{xnm}0", name=f"x{xnm}0")
        if xnm == "k":
            nc.sync.dma_start(t0[:, 0:NF // 2], io["xk"][:, 0:NF // 2, :, 0:QC])
            nc.sync.dma_start(t0[:, NF // 2:], io["xk"][:, NF // 2:, :, 0:QC])
        else:
            nc.sync.dma_start(t0[:], io["x" + xnm][:, :, :, 0:QC])
        bx0[xnm] = t0
    wq_sb, wk_sb, wv_sb = w_sbs["wq"], w_sbs["wk"], w_sbs["wv"]
    tri_sb = consts.tile([P, P], F16, tag="tri", name="tri_sb")
    nc.sync.dma_start(tri_sb[:], io["tri"])
    eye_sb = consts.tile([P, P], F16, tag="eye", name="eye_sb")
    nc.sync.dma_start(eye_sb[:], io["eye"])
    for nm in ("q", "k", "v"):
        bxr[nm] = xs.tile([P, NF, nsel, S - QC], xdt, tag=f"x{nm}r",
                          name=f"x{nm}r")
    # per-chunk input DMAs, interleaved q/k/v, in consumption order
    for j in range(1, NCH):
        for nm in ("q", "k", "v"):
            nc.sync.dma_start(
                bxr[nm][:, :, :, (j - 1) * QC:j * QC],
                io["x" + nm][:, :, :, j * QC:(j + 1) * QC])
    wo_sb = consts.tile([P, 2, NF, P], F16, tag="wo", name="wo_sb")
    nc.sync.dma_start(wo_sb[:], io["wo"])

    # PE p-state warmup: the startup is DMA-bound; dependency-free dummy
    # matmuls keep the tensor engine clock ramped so the first real
    # projections run at full speed instead of the mid-p-state 2x penalty.
    warm = consts.tile([P, QC], F16, tag="warm", name="warm")
    nc.vector.memset(warm[:], 0.0)
    wps = pspool.tile([P, QC], F32, tag="aux", name="warmps", bufs=2)
    for _ in range(20):
        nc.tensor.matmul(wps[:], warm[:, 0:P], warm[:],
                         start=True, stop=True, skip_group_check=True)

    def xsl(nm, lo, hi):
        """[P, NF, nsel, hi-lo] view of input nm tokens [lo:hi)."""
        if hi <= QC:
            return bx0[nm][:, :, :, lo:hi]
        return bxr[nm][:, :, :, lo - QC:hi - QC]

    def ps_tile(tag, shape=None, dtype=F32, bufs=None):
        shape = shape or [P, QC]
        return pspool.tile(shape, dtype, tag=tag, name=tag, bufs=bufs)

    # ---- projection units (queued as PE fillers) -------------------------
    def mk_proj_qk_unit(j, w_sb, b_sb, out, t, srcnm):
        """One psum tile of a Q/K projection: matmuls + biased fp16 evict."""
        co = j * QC

        def go():
            pp = ps_tile("aux", bufs=2)
            if SPLIT_PROJ:
                first = True
                for hh in range(2):  # token halves (moving free <= 512)
                    ts0, ts1 = co + hh * 256, co + (hh + 1) * 256
                    for f in range(0, NF, 2):  # main terms: hi x hi pairs
                        nc.tensor.matmul(
                            pp[:, hh * 256:(hh + 1) * 256],
                            w_sb[:, f:f + 2, 0, t * P:(t + 1) * P],
                            xsl(srcnm, ts0, ts1)[:, f:f + 2, 1, :],
                            start=first, stop=False, perf_mode=DR,
                            skip_group_check=True)
                        first = False
                    for f in range(NF):  # cross terms: (hi,lo)x(lo,hi)
                        nc.tensor.matmul(
                            pp[:, hh * 256:(hh + 1) * 256],
                            w_sb[:, f, 0:2, t * P:(t + 1) * P],
                            xsl(srcnm, ts0, ts1)[:, f, 0:2, :],
                            start=False, stop=(f == NF - 1 and hh == 1),
                            perf_mode=DR, skip_group_check=True)
                scl = 1.0 / WSCALE
            else:
                for f in range(NF):
                    nc.tensor.matmul(
                        pp[:], w_sb[:, f, 0, t * P:(t + 1) * P],
                        xsl(srcnm, co, co + QC)[:, f, 0, :],
                        start=(f == 0), stop=(f == NF - 1))
                scl = 1.0
            nc.vector.tensor_scalar(out[:, t, :], pp[:], scl,
                                    b_sb[:, t:t + 1], MUL, ADD)
        return go

    def mk_proj_v_unit(j, t4):
        """One 128-token tile of the V projection -> vt tile (token-major)."""
        co = j * QC

        def go():
            pp = ps_tile("aux", [P, DKC], bufs=2)
            ts0, ts1 = co + t4 * P, co + (t4 + 1) * P
            if SPLIT_PROJ:
                first = True
                for f in range(0, NF, 2):
                    nc.tensor.matmul(
                        pp[:], xsl("v", ts0, ts1)[:, f:f + 2, 1, :],
                        wv_sb[:, f:f + 2, 0, :],
                        start=first, stop=False, perf_mode=DR,
                        skip_group_check=True)
                    first = False
                for f in range(NF):
                    nc.tensor.matmul(
                        pp[:], xsl("v", ts0, ts1)[:, f, 0:2, :],
                        wv_sb[:, f, 0:2, :],
                        start=False, stop=(f == NF - 1),
                        perf_mode=DR, skip_group_check=True)
                scl = 1.0 / WSCALE
            else:
                for f in range(NF):
                    nc.tensor.matmul(
                        pp[:], xsl("v", ts0, ts1)[:, f, 0, :],
                        wv_sb[:, f, 0, :],
                        start=(f == 0), stop=(f == NF - 1))
                scl = 1.0
            kt = j * KPC + t4
            vt = vt_tiles[kt]
            # ones column: PV's column 64 accumulates the softmax sums
            nc.vector.memset(vt[:, :, 64:65], 1.0)
            nc.vector.tensor_scalar_mul(
                vt[:, :, 0:64], pp[:].rearrange("p (h d) -> p h d", d=64), scl)
        return go

    qt = {}
    kt_tiles = {}
    vt_tiles = {}
    # cross-segment software pipeline state: the previous event's PV
    # emission and the previous segment's normalize hook
    pending = {"pv": None, "post": None, "tick": 0}

    # Filler queues of PE closures drained inside ACT-bound attention
    # k-loops: `proj_fillers` (next chunk's projections + ctx transposes —
    # must drain promptly) and `wo_fillers` (Wo matmuls + output DMA of
    # finished chunks — deferred into the LAST chunk's long k-loop, which
    # is the only ACT-bound phase with spare PE time).
    proj_fillers = []
    wo_fillers = []

    def drain_filler(n=1, wo_ok=False):
        for _ in range(n):
            if proj_fillers:
                proj_fillers.pop(0)()
            elif wo_ok and wo_fillers:
                wo_fillers.pop(0)()

    def queue_proj_qk(jn):
        for (w_sb, b_sb, tag, srcnm) in ((wq_sb, bq_sb, f"qt{jn % 2}", "q"),
                                         (wk_sb, bk_sb, f"kt{jn}", "k")):
            out = persist.tile([P, 2, QC], F16, tag=tag, name=tag)
            if srcnm == "q":
                qt[jn % 2] = out
            else:
                kt_tiles[jn] = out
            for t in range(2):
                proj_fillers.append(
                    mk_proj_qk_unit(jn, w_sb, b_sb, out, t, srcnm))

    def queue_proj_v(jn):
        for t4 in range(KPC):
            kt = jn * KPC + t4
            vt_tiles[kt] = persist.tile([P, HPC, 65], F16, tag=f"vt{kt}",
                                        name=f"vt{kt}")
            proj_fillers.append(mk_proj_v_unit(jn, t4))

    def queue_proj(jn):
        queue_proj_qk(jn)
        queue_proj_v(jn)

    # ---- attention pass (2 heads) ---------------------------------------
    def emit_pass(j, p):
        """Attention for chunk j, heads {2p, 2p+1}. Returns pvacc tiles."""
        nkt = KPC * (j + 1)
        qtile = qt[j % 2]
        pv = [ps_tile(f"pv{l}", [P, KPC, 80]) for l in range(2)]

        def head_slices(l):
            """(dim-tile t, partition offset) for local head l of pass p."""
            hh = 2 * p + l
            return hh // 2, (hh % 2) * 64

        def emit_qk_exp(kt):
            jk = kt // KPC          # source chunk of this k-tile
            ko = (kt % KPC) * P
            tdiag = kt - KPC * j    # >=0 on the diagonal chunk
            ktile = kt_tiles[jk]
            sw = ps_tile("swA" if kt % 2 == 0 else "swB", [P, 2 * QC])
            c0 = max(tdiag, 0) * P
            for l in range(2):
                t, po = head_slices(l)
                nc.tensor.matmul(
                    sw[:, l * QC + c0:(l + 1) * QC],
                    ktile[po:po + 64, t, ko:ko + P],
                    qtile[po:po + 64, t, c0:QC],
                    start=True, stop=True)
            pt = pts.tile([P, 2 * QC], F16, tag="pt", name="pt")
            if c0:
                # one strided exp covering both heads' live columns
                swv = sw[:].rearrange("p (l q) -> p l q", l=2)[:, :, c0:QC]
                ptv = pt[:].rearrange("p (l q) -> p l q", l=2)[:, :, c0:QC]
                nc.scalar.activation(ptv, swv, Exp, scale=0.125)
            else:
                nc.scalar.activation(pt[:], sw[:], Exp, scale=0.125)
            if tdiag >= 0:
                # masked-diagonal multiply on the (otherwise idle) Pool
                # engine so exp-dependent work never blocks DVE's evictions
                for l in range(2):
                    nc.gpsimd.tensor_mul(pt[:, l * QC + c0:l * QC + c0 + P],
                                         pt[:, l * QC + c0:l * QC + c0 + P],
                                         tri_sb[:])
            if DEBUG and j == 0 and p == 0 and kt == 0:
                nc.sync.dma_start(io["dbg_pt"], pt[:])

            def emit_pv():
                vt = vt_tiles[kt]
                for l in range(2):
                    hh = 2 * p + l
                    for t in range(max(tdiag, 0), KPC):
                        nc.tensor.matmul(
                            pv[l][:, t, 0:65],
                            pt[:, l * QC + t * P:l * QC + (t + 1) * P],
                            vt[:, hh, :],
                            start=(kt == 0 and t == 0),
                            stop=(kt == KPC * j + t),
                            skip_group_check=True)
            return emit_pv

        # Globally software-pipelined: the PV of the previous event — which
        # may belong to the PREVIOUS segment — is emitted after this event's
        # QK+exp, so the ACT engine never waits at segment boundaries. The
        # previous segment's normalize hook runs right after its last PV.
        wo_ok = (j >= NCH - 2)
        for kt in range(nkt):
            pv_next = emit_qk_exp(kt)
            if pending["pv"] is not None:
                pending["pv"]()
            if pending["post"] is not None:
                pending["post"]()
                pending["post"] = None
            drain_filler(2 if len(proj_fillers) > 8 else 1, wo_ok)
            pending["pv"] = pv_next
        return pv

    # ---- normalize + transpose + Wo for one chunk ------------------------
    def emit_norm(j, p, pv):
        """Normalize pass p of chunk j into ctx_sb (DVE), return ctx_sb."""
        cs = persist.tile([P, 2, KPC, 64], F16, tag=f"ctxs{p}",
                          name=f"ctxs{p}")
        for l in range(2):
            rc = persist.tile([P, KPC], F32, tag=f"rc{p}{l}", name=f"rc{p}{l}")
            nc.vector.reciprocal(rc[:], pv[l][:, :, 64:65].squeeze(2))
            for t in range(KPC):
                nc.vector.tensor_scalar_mul(cs[:, l, t, :], pv[l][:, t, 0:64],
                                            rc[:, t:t + 1])
        return cs

    def mk_transp(cs, ctx_t, p):
        def go():
            trp = ps_tile("aux", [P, QC], F16, bufs=2)
            for l in range(2):
                for t in range(KPC):
                    nc.tensor.matmul(
                        trp[l * 64:(l + 1) * 64, t * P:(t + 1) * P],
                        cs[:, l, t, :], eye_sb[:],
                        is_transpose=True,
                        start=(t == 0), stop=(t == KPC - 1),
                        skip_group_check=True,
                        tile_position=(0, l * 64))
            nc.vector.tensor_copy(ctx_t[:, p, :], trp[:])
        return go

    def mk_wo(ctx_t, wout, m, act_ok, t_only=None):
        def go():
            if act_ok:
                # tail of the last chunk: score banks are free, use them to
                # deepen the psum rotation so Wo/evict fully pipeline
                tag = ("aux", "swA", "swB")[m % 3]
            else:
                tag = "aux"
            po = ps_tile(tag, [P, QC], bufs=2 if tag == "aux" else None)
            ts = (0, 1) if t_only is None else (t_only,)
            for i, t in enumerate(ts):
                nc.tensor.matmul(po[:], wo_sb[:, t, m, :], ctx_t[:, t, :],
                                 start=(i == 0), stop=(i == len(ts) - 1))
            if act_ok and m % 2:
                # tail chunk: ACT is idle there, share the eviction load
                nc.scalar.copy(wout[:, m, :], po[:])
            else:
                nc.vector.tensor_copy(wout[:, m, :], po[:])
        return go

    def mk_out_dma(j, wout, m0, m1, dst="out_t"):
        def go():
            nc.gpsimd.dma_start(
                io[dst].rearrange("(m p) s -> p m s", p=P)
                [:, m0:m1, j * QC:(j + 1) * QC], wout[:, m0:m1, :])
        return go

    # ---- main loop -------------------------------------------------------
    # chunk 0's projections run up front; chunk j+1's are drained as PE
    # fillers inside chunk j's ACT-bound attention, together with chunk
    # j-1/j's transpose + Wo + output-DMA epilogue.
    queue_proj_qk(0)
    while proj_fillers:
        proj_fillers.pop(0)()
    queue_proj_v(0)
    for j in range(NCH):
        if j + 1 < NCH:
            queue_proj(j + 1)
        ctx_t = persist.tile([P, 2, QC], F16, tag=f"ctxt{j}",
                             name=f"ctxt{j}")
        wout = wouts.tile([P, NF, QC], F16, tag="wout", name="wout")
        for p in range(2):
            pv = emit_pass(j, p)

            def mk_post(j=j, p=p, pv=pv, ctx_t=ctx_t, wout=wout):
                last = (j == NCH - 1)

                def post():
                    cs = emit_norm(j, p, pv)
                    proj_fillers.append(mk_transp(cs, ctx_t, p))
                    if last and p == 0:
                        # last chunk: pass-0 half of Wo runs during pass 1's
                        # k-loop, into a second output summed on the host
                        w2 = wouts.tile([P, NF, QC], F16, tag="wout",
                                        name="wout2")
                        for m in range(NF):
                            wo_fillers.append(
                                mk_wo(ctx_t, w2, m, False, t_only=0))
                            if m == NF // 2 - 1:
                                wo_fillers.append(
                                    mk_out_dma(j, w2, 0, NF // 2, "out_t2"))
                        wo_fillers.append(
                            mk_out_dma(j, w2, NF // 2, NF, "out_t2"))
                    elif p == 1:
                        t_only = 1 if last else None
                        for m in range(NF):
                            wo_fillers.append(
                                mk_wo(ctx_t, wout, m, last, t_only=t_only))
                            if m == NF // 2 - 1:
                                wo_fillers.append(
                                    mk_out_dma(j, wout, 0, NF // 2))
                        wo_fillers.append(mk_out_dma(j, wout, NF // 2, NF))
                return post
            pending["post"] = mk_post()
        # chunk j+1's attention reads qt/kt/vt(j+1): those projections must
        # be fully emitted before the next emit_pass
        while proj_fillers:
            proj_fillers.pop(0)()
    if pending["pv"] is not None:
        pending["pv"]()
        pending["pv"] = None
    if pending["post"] is not None:
        pending["post"]()
        pending["post"] = None
    while proj_fillers:
        proj_fillers.pop(0)()
    while wo_fillers:
        wo_fillers.pop(0)()


def build_program():
    import concourse.tile as tile
    from concourse import bacc, mybir
    from contextlib import ExitStack

    F8 = mybir.dt.float8e4
    F16 = mybir.dt.float16
    F32 = mybir.dt.float32

    nc = bacc.Bacc("TRN2", target_bir_lowering=False, debug=False)
    xdt = F8 if SPLIT_PROJ else F16
    nsel = 2 if SPLIT_PROJ else 1
    wdt = F8 if SPLIT_PROJ else F16
    io = {
        "xq": nc.dram_tensor("xq", [P, NF, nsel, S], xdt,
                             kind="ExternalInput").ap(),
        "xk": nc.dram_tensor("xk", [P, NF, nsel, S], xdt,
                             kind="ExternalInput").ap(),
        "xv": nc.dram_tensor("xv", [P, NF, nsel, S], xdt,
                             kind="ExternalInput").ap(),
        "wq": nc.dram_tensor("wq", [P, NF, nsel, DKC], wdt,
                             kind="ExternalInput").ap(),
        "wk": nc.dram_tensor("wk", [P, NF, nsel, DKC], wdt,
                             kind="ExternalInput").ap(),
        "wv": nc.dram_tensor("wv", [P, NF, nsel, DKC], wdt,
                             kind="ExternalInput").ap(),
        "wo": nc.dram_tensor("wo", [P, 2, NF, P], F16,
                             kind="ExternalInput").ap(),
        "tri": nc.dram_tensor("tri", [P, P], F16, kind="ExternalInput").ap(),
        "eye": nc.dram_tensor("eye", [P, P], F16, kind="ExternalInput").ap(),
        "bq": nc.dram_tensor("bq", [P, 2], F32, kind="ExternalInput").ap(),
        "bk": nc.dram_tensor("bk", [P, 2], F32, kind="ExternalInput").ap(),
        "out_t": nc.dram_tensor("out_t", [D, S], F16,
                                kind="ExternalOutput").ap(),
        "out_t2": nc.dram_tensor("out_t2", [D, S], F16,
                                 kind="ExternalOutput").ap(),
    }
    if DEBUG:
        io["dbg_qt"] = nc.dram_tensor("dbg_qt", [P, 2, QC], F16,
                                      kind="ExternalOutput").ap()
        io["dbg_kt"] = nc.dram_tensor("dbg_kt", [P, 2, QC], F16,
                                      kind="ExternalOutput").ap()
        io["dbg_vt"] = nc.dram_tensor("dbg_vt", [P, HPC, 65], F16,
                                      kind="ExternalOutput").ap()
        io["dbg_pt"] = nc.dram_tensor("dbg_pt", [P, 2 * QC], F16,
                                      kind="ExternalOutput").ap()
        io["dbg_ctxs"] = nc.dram_tensor("dbg_ctxs", [P, 2, KPC, 64], F16,
                                        kind="ExternalOutput").ap()
    with tile.TileContext(nc) as tc, ExitStack() as ctx:
        _mha_body(ctx, tc, io)
    nc.compile()
    return nc


# ---------------------------------------------------------------------------
# Host side
# ---------------------------------------------------------------------------

def _np_reference(query, key, value, mask, Wq, bq, Wk, bk, Wv, bv, Wo, bo):
    q = (query.reshape(-1, D) @ Wq + bq).reshape(B, S, H, DK).transpose(0, 2, 1, 3)
    k = (key.reshape(-1, D) @ Wk + bk).reshape(B, S, H, DK).transpose(0, 2, 1, 3)
    v = (value.reshape(-1, D) @ Wv + bv).reshape(B, S, H, DK).transpose(0, 2, 1, 3)
    scores = np.einsum("bhqd,bhkd->bhqk", q, k) / math.sqrt(DK)
    scores = np.where(mask[:, None, :, :] == 0, np.float32(-1e9), scores)
    scores -= scores.max(axis=-1, keepdims=True)
    p = np.exp(scores)
    p /= p.sum(axis=-1, keepdims=True)
    x = np.einsum("bhqk,bhkd->bhqd", p, v)
    x = x.transpose(0, 2, 1, 3).reshape(B, -1, D)
    return (x @ Wo + bo).astype(np.float32)


def _split8(x):
    """fp32 -> (lo, hi) fp8e4 with hi = fp8(x), lo = fp8(x - hi)."""
    hi = x.astype(F8NP)
    lo = (x - hi.astype(np.float32)).astype(F8NP)
    return lo, hi


def _xlayout(x):
    """[S, D] batch slice -> [128, NF, nsel, S]: [p, f, sel, t] with
    sel = (lo, hi) fp8 split (or a single fp16 plane)."""
    xt = np.ascontiguousarray(x.T)  # [D, S]
    xt = xt.reshape(NF, P, S).transpose(1, 0, 2)  # [p, f, t]
    if not SPLIT_PROJ:
        return np.ascontiguousarray(xt[:, :, None, :]).astype(np.float16)
    lo, hi = _split8(xt.astype(np.float32))
    out = np.empty((P, NF, 2, S), F8NP)
    out[:, :, 0, :] = lo
    out[:, :, 1, :] = hi
    return out


def _wlayout(w):
    """[D, DKC] weight slice -> [128, NF, nsel, DKC]: [p, f, sel, m] with
    sel = (hi, lo) of the x32-scaled weight (or single fp16 plane)."""
    wr = w.reshape(NF, P, DKC).transpose(1, 0, 2)  # [p, f, m]
    if not SPLIT_PROJ:
        return np.ascontiguousarray(wr[:, :, None, :]).astype(np.float16)
    lo, hi = _split8(wr.astype(np.float32) * WSCALE)
    out = np.empty((P, NF, 2, DKC), F8NP)
    out[:, :, 0, :] = hi
    out[:, :, 1, :] = lo
    return out


def _shard_inputs(query, key, value, Wq, bq, Wk, bk, Wv, Wo):
    idx = np.arange(P)
    tri = (idx[:, None] <= idx[None, :]).astype(np.float16)  # [k, q] k<=q
    eye = np.eye(P, dtype=np.float16)
    in_maps = []
    xcache = {}
    for c in range(NCORES):
        bb, g = c // 4, c % 4
        sl = slice(g * DKC, (g + 1) * DKC)
        if bb not in xcache:
            xcache[bb] = {
                "xq": _xlayout(query[bb]),
                "xk": _xlayout(key[bb]),
                "xv": _xlayout(value[bb]),
            }
        wo = Wo[sl, :].reshape(2, P, NF, P).transpose(1, 0, 2, 3)
        in_maps.append({
            **xcache[bb],
            "wq": _wlayout(Wq[:, sl]),
            "wk": _wlayout(Wk[:, sl]),
            "wv": _wlayout(Wv[:, sl]),
            "wo": np.ascontiguousarray(wo).astype(np.float16),
            "tri": tri,
            "eye": eye,
            "bq": np.ascontiguousarray(bq[sl].reshape(2, P).T).astype(np.float32),
            "bk": np.ascontiguousarray(bk[sl].reshape(2, P).T).astype(np.float32),
        })
    return in_maps


def kernel(**inputs):
    query = np.asarray(inputs["query"], np.float32)
    key = np.asarray(inputs["key"], np.float32)
    value = np.asarray(inputs["value"], np.float32)
    mask = np.asarray(inputs["mask"])
    Wq = np.asarray(inputs["Wq"], np.float32)
    bq = np.asarray(inputs["bq"], np.float32)
    Wk = np.asarray(inputs["Wk"], np.float32)
    bk = np.asarray(inputs["bk"], np.float32)
    Wv = np.asarray(inputs["Wv"], np.float32)
    bv = np.asarray(inputs["bv"], np.float32)
    Wo = np.asarray(inputs["Wo"], np.float32)
    bo = np.asarray(inputs["bo"], np.float32)

    tril = np.tril(np.ones((S, S), np.int8))
    if mask.shape != (B, S, S) or not np.array_equal(
            (mask != 0).astype(np.int8), np.broadcast_to(tril, (B, S, S))):
        return _np_reference(query, key, value, mask,
                             Wq, bq, Wk, bk, Wv, bv, Wo, bo)

    in_maps = _shard_inputs(query, key, value, Wq, bq, Wk, bk, Wv, Wo)
    outs = _run_spmd(in_maps)  # [8, D, S]

    outs = outs.astype(np.float32)
    corr = (bv @ Wo + bo)[None, :]
    res = np.empty((B, S, D), np.float32)
    for bb in range(B):
        acc = outs[bb * 4:(bb + 1) * 4].sum(axis=0)  # [D, S]
        res[bb] = acc.T + corr
    return res


def _get_exec():
    if "exec" in _PROGRAM_CACHE:
        return _PROGRAM_CACHE["exec"]
    import jax
    from jax.sharding import Mesh, PartitionSpec
    from jax.experimental.shard_map import shard_map
    import concourse.mybir as mybir
    from concourse import bass2jax

    nc = build_program()
    _PROGRAM_CACHE["nc"] = nc
    bass2jax.install_neuronx_cc_hook()
    partition_name = nc.partition_id_tensor.name if nc.partition_id_tensor else None
    in_names, out_names, out_avals, zero_outs = [], [], [], []
    for alloc in nc.m.functions[0].allocations:
        if not isinstance(alloc, mybir.MemoryLocationSet):
            continue
        name = alloc.memorylocations[0].name
        if alloc.kind == "ExternalInput":
            if name != partition_name:
                in_names.append(name)
        elif alloc.kind == "ExternalOutput":
            out_names.append(name)
            shape = tuple(alloc.tensor_shape)
            dtype = mybir.dt.np(alloc.dtype)
            out_avals.append(jax.core.ShapedArray(shape, dtype))
            zero_outs.append(np.zeros(shape, dtype))
    n_params = len(in_names)
    all_in_names = list(in_names) + list(out_names)
    if partition_name is not None:
        all_in_names.append(partition_name)

    def _body(*args):
        operands = list(args)
        if partition_name is not None:
            operands.append(bass2jax.partition_id_tensor())
        return tuple(bass2jax._bass_exec_p.bind(
            *operands,
            out_avals=tuple(out_avals),
            in_names=tuple(all_in_names),
            out_names=tuple(out_names),
            lowering_input_output_aliases=(),
            sim_require_finite=True,
            sim_require_nnan=True,
            nc=nc,
        ))

    devices = jax.devices()[:NCORES]
    assert len(devices) >= NCORES, f"need {NCORES} cores, have {len(devices)}"
    mesh = Mesh(np.asarray(devices[:NCORES]), ("core",))
    fn = jax.jit(
        shard_map(_body, mesh=mesh,
                  in_specs=(PartitionSpec("core"),) * (n_params + len(zero_outs)),
                  out_specs=(PartitionSpec("core"),) * len(out_names),
                  check_rep=False),
        donate_argnums=tuple(range(n_params, n_params + len(out_names))),
        keep_unused=True)
    _PROGRAM_CACHE["exec"] = (fn, in_names, zero_outs)
    return _PROGRAM_CACHE["exec"]


def _run_spmd(in_maps):
    fn, in_names, zero_outs = _get_exec()
    concat_in = [np.concatenate([np.asarray(in_maps[c][nm])
                                 for c in range(NCORES)], axis=0)
                 for nm in in_names]
    concat_zero = [np.zeros((NCORES * z.shape[0], *z.shape[1:]), z.dtype)
                   for z in zero_outs]
    out = fn(*concat_in, *concat_zero)
    LAST["out"] = out
    acc = np.asarray(out[0]).astype(np.float32)
    for o in out[1:]:
        acc = acc + np.asarray(o).astype(np.float32)
    return acc.reshape(NCORES, D, S)
